# revision 13
# baseline (speedup 1.0000x reference)
"""Trainium2 Bass kernel for the L1 tensor-product problem.

Math (per batch row b):
  out0e = [x0e*s, CG*(x1o.v)] @ W0e * NORM0E
  out0o = [x0o*s, CG*(x1e.v)] @ W0o * NORM0O
  out1e_c = [CG*x0o*v_c, CG*x1e_c*s, CGC*cross(x1o,v)_c] @ W1e * NORM1E
  out1o_c = [CG*x0e*v_c, CG*x1o_c*s, CGC*cross(x1e,v)_c] @ W1o * NORM1O

Kernel strategy (pure data parallel over batch, 8 cores), v5:
  * bf16 wire + matmul dtype; PSUM accumulates fp32 (rel-err ~4e-3,
    budget 2e-2).
  * DVE is the bottleneck engine (~100% busy): all per-row products run
    as 2x-mode bf16 tensor_tensor ops at the hw max of ~1.92 elem/ns,
    and the schedule keeps DVE *elements* at the formulation's minimum
    (38 chunks per tile-column):
      - pvs: ONE 24T op computes x1 x {v0, v1, v2, s} in an [m,k,t]
        layout: x1e*s / x1o*s land contiguous for the h-path matmuls
        and diag/off-diag blocks at AP-addressable offsets for the
        k+/k- matmuls and the dots.
      - dots (2 adds, paired across parities), t3 = v_c*g (6T).
  * Unscaled g = x0?' @ Wg at the head of each PE seg so sgp is ready
    a full seg before the t3 op that consumes it.
  * First/last tile are split into 4 W=128 subtiles so the pipeline
    ramp (first load -> first DVE op) and the tail drain chain are a
    quarter-length.  Edge wedges are packed CONTIGUOUSLY on the host
    (xe/ye/s4e params) -- slicing wedges out of the tile-major layout
    shatters the DMA into 256B packets (measured 40k packets, DMA 86%
    busy, DVE starved at 80%).
  * All DMA in/out APs are 2D contiguous slices; each partition's
    tile-load is one 10KB descriptor run.
  * Multiplier rows (v,s) reach all 128 partitions via a stride-0
    broadcast DMA read (GpSimd is unusable: its SBUF port contends
    with 2-port DVE ops and its tensor ops trip the chip's utilization
    throttle; PE ones-broadcasts would eat the PE headroom).
  * PSUM accumulation is slice-major (see memory: interleaving
    start/stop groups across slices of one psum region is wrong on hw).
"""

import sys

sys.path.insert(0, "/opt/trn_rl_repo")

import numpy as np

import concourse.bass as bass
import concourse.bacc as bacc
import concourse.mybir as mybir
from concourse.bass_utils import run_bass_kernel_spmd
from concourse.tile import TileContext

N_CORES = 8
T = 512  # batch columns per full tile
WEDGE = 128  # subtile width for first/last tile
NW = T // WEDGE  # wedges per edge tile

# irreps: 256x0e + 256x0o + 128x1e + 128x1o
CG = 1.0 / 3.0**0.5
CGC = 1.0 / 6.0**0.5
NORM0E = (1.0 / 384.0) ** 0.5
NORM0O = (1.0 / 384.0) ** 0.5
NORM1E = (3.0 / 512.0) ** 0.5
NORM1O = (3.0 / 512.0) ** 0.5

_BF16 = None


def _bf16():
    global _BF16
    if _BF16 is None:
        import ml_dtypes

        _BF16 = np.dtype(ml_dtypes.bfloat16)
    return _BF16


def _pack_weights(W0e, W0o, W1e, W1o):
    """Fold constants/signs; 22 lhsT chunks [128,128] side by side.

    Order: 0e (kc0m0,kc0m1,kc1m0,kc1m1,kc2m0,kc2m1), 0o (same 6),
    1e (g0,g1,h,k+,k-), 1o (g0,g1,h,k+,k-), identity.
    """
    W0e = W0e.astype(np.float64) * NORM0E
    W0e[256:] *= CG
    W0o = W0o.astype(np.float64) * NORM0O
    W0o[256:] *= CG
    W1e = W1e.astype(np.float64) * NORM1E
    W1e[:384] *= CG
    W1e[384:] *= CGC
    W1o = W1o.astype(np.float64) * NORM1O
    W1o[:384] *= CG
    W1o[384:] *= CGC
    chunks = []
    for W in (W0e, W0o):  # [384, 256]
        for kc in range(3):
            for mc in range(2):
                chunks.append(W[kc * 128 : (kc + 1) * 128, mc * 128 : (mc + 1) * 128])
    for W in (W1e, W1o):  # [512, 128]
        chunks.append(W[0:128, :])      # g0
        chunks.append(W[128:256, :])    # g1
        chunks.append(W[256:384, :])    # h
        chunks.append(W[384:512, :])    # k+
        chunks.append(-W[384:512, :])   # k-
    chunks.append(np.eye(128, dtype=np.float64))  # 22: identity (combine accum)
    packed = np.concatenate(chunks, axis=1)
    return np.ascontiguousarray(packed.astype(_bf16()))


def _pack_rows(in1_r, in2_r, Wd):
    """Pack a block of Wd rows: -> x [128, 10*Wd] bf16, s4 [4, Wd] bf16.

    Chunk order: 0,1=x0e  2,3=x0o  4+c=x1e_c  7+c=x1o_c.
    Multiplier rows in [v0, v1, v2, s] order.
    """
    dt = _bf16()
    x = np.empty((128, 10, Wd), dt)
    x[:, 0:4] = in1_r[:, 0:512].reshape(Wd, 4, 128).transpose(2, 1, 0)
    x[:, 4:7] = in1_r[:, 512:896].reshape(Wd, 128, 3).transpose(1, 2, 0)
    x[:, 7:10] = in1_r[:, 896:1280].reshape(Wd, 128, 3).transpose(1, 2, 0)
    s4 = np.ascontiguousarray(in2_r[:, [1, 2, 3, 0]].T.astype(dt))
    return np.ascontiguousarray(x.reshape(128, 10 * Wd)), s4


def _prep_shard(in1_s, in2_s):
    """Middle tiles tile-major + edge wedges contiguous."""
    Bs = in1_s.shape[0]
    nt = Bs // T
    dt = _bf16()
    if nt == 1:
        nmid = 0
        edges = [(j * WEDGE, WEDGE) for j in range(NW)]
    else:
        nmid = nt - 2
        edges = [(j * WEDGE, WEDGE) for j in range(NW)] + [
            (Bs - T + j * WEDGE, WEDGE) for j in range(NW)
        ]
    xm = np.empty((max(nmid, 1), 128, 10 * T), dt)
    s4m = np.empty((max(nmid, 1), 4, T), dt)
    for t in range(nmid):
        r = slice(T + t * T, T + (t + 1) * T)
        xm[t], s4m[t] = _pack_rows(in1_s[r], in2_s[r], T)
    ne = len(edges)
    xe = np.empty((ne, 128, 10 * WEDGE), dt)
    s4e = np.empty((ne, 4, WEDGE), dt)
    for j, (off, Wd) in enumerate(edges):
        xe[j], s4e[j] = _pack_rows(in1_s[off : off + Wd], in2_s[off : off + Wd], Wd)
    return xm, s4m, xe, s4e


def _unpack_block(yb, Wd):
    """[128, 10*Wd] bf16 -> [Wd, 1280] fp32."""
    yb = np.asarray(yb).reshape(128, 10, Wd).astype(np.float32)
    out = np.empty((Wd, 1280), np.float32)
    out[:, 0:512] = yb[:, 0:4].transpose(2, 1, 0).reshape(Wd, 512)
    out[:, 512:896] = yb[:, 4:7].transpose(2, 0, 1).reshape(Wd, 384)
    out[:, 896:1280] = yb[:, 7:10].transpose(2, 0, 1).reshape(Wd, 384)
    return out


def _post_shard(ym, ye, Bs):
    nt = Bs // T
    out = np.empty((Bs, 1280), np.float32)
    if nt == 1:
        for j in range(NW):
            out[j * WEDGE : (j + 1) * WEDGE] = _unpack_block(ye[j], WEDGE)
        return out
    for t in range(nt - 2):
        out[T + t * T : T + (t + 1) * T] = _unpack_block(ym[t], T)
    for j in range(NW):
        out[j * WEDGE : (j + 1) * WEDGE] = _unpack_block(ye[j], WEDGE)
        out[Bs - T + j * WEDGE : Bs - T + (j + 1) * WEDGE] = _unpack_block(
            ye[NW + j], WEDGE
        )
    return out


def _build_program(Bs):
    assert Bs % T == 0, (Bs, T)
    nt = Bs // T
    nmid = 0 if nt == 1 else nt - 2
    ne = NW if nt == 1 else 2 * NW
    bf = mybir.dt.bfloat16
    f32 = mybir.dt.float32

    nc = bacc.Bacc()
    x = nc.declare_dram_parameter("x", [max(nmid, 1), 128, 10 * T], bf, isOutput=False)
    s4 = nc.declare_dram_parameter("s4", [max(nmid, 1), 4, T], bf, isOutput=False)
    xe = nc.declare_dram_parameter("xe", [ne, 128, 10 * WEDGE], bf, isOutput=False)
    s4e = nc.declare_dram_parameter("s4e", [ne, 4, WEDGE], bf, isOutput=False)
    w = nc.declare_dram_parameter("w", [128, 23 * 128], bf, isOutput=False)
    y = nc.declare_dram_parameter("y", [max(nmid, 1), 128, 10 * T], bf, isOutput=True)
    ye = nc.declare_dram_parameter("ye", [ne, 128, 10 * WEDGE], bf, isOutput=True)

    # segment order: leading wedges, middle tiles, trailing wedges
    segs = [("e", j, WEDGE) for j in range(NW)]
    segs += [("m", t, T) for t in range(nmid)]
    if nt > 1:
        segs += [("e", NW + j, WEDGE) for j in range(NW)]
    nseg = len(segs)

    with TileContext(nc) as tc:
        with (
            tc.tile_pool(name="wpool", bufs=1) as wpool,
            tc.tile_pool(name="xpool", bufs=3) as xpool,
            tc.tile_pool(name="mbpool", bufs=3) as mbpool,
            tc.tile_pool(name="pvpool", bufs=3) as pvpool,
            tc.tile_pool(name="pspool", bufs=3) as pspool,
            tc.tile_pool(name="cpool", bufs=2) as cpool,
            tc.tile_pool(name="ypool", bufs=2) as ypool,
            tc.tile_pool(name="psum", bufs=8, space="PSUM") as psum,
        ):
            wt = wpool.tile([128, 23 * 128], bf)

            def W(i):
                return wt[:, i * 128 : (i + 1) * 128]

            def load(seg):
                kind, t, Wd = seg
                xd = x[t] if kind == "m" else xe[t]
                sd = s4[t] if kind == "m" else s4e[t]
                yd = y[t] if kind == "m" else ye[t]
                mbt = mbpool.tile([128, 4 * T], bf, tag="mb", name="mb_t")[
                    :, : 4 * Wd
                ]
                nc.sync.dma_start(
                    out=mbt.rearrange("p (c t) -> p c t", c=4),
                    in_=sd.unsqueeze(0).broadcast_to([128, 4, Wd]),
                )
                xt = xpool.tile([128, 10 * T], bf, tag="xt", name="x_t")[
                    :, : 10 * Wd
                ]
                # upper 6 chunks first: pvs (the big DVE op) needs only these
                nc.sync.dma_start(out=xt[:, 4 * Wd :], in_=xd[:, 4 * Wd :])
                nc.sync.dma_start(out=xt[:, : 4 * Wd], in_=xd[:, : 4 * Wd])
                return {"xt": xt, "mbt": mbt, "W": Wd, "yd": yd}

            def mm_into(p, contribs, first=True, last=True):
                n = len(contribs)
                for i, (wi, rhs) in enumerate(contribs):
                    nc.tensor.matmul(
                        p,
                        W(wi),
                        rhs,
                        start=(first and i == 0),
                        stop=(last and i == n - 1),
                    )

            def stage_g(st):
                # g = x0?' @ Wg (unscaled; only needs xt).  Emitted at the
                # head of each PE seg so sgp is ready a full seg before the
                # t3 op that consumes it.
                xt, Wd = st["xt"], st["W"]
                sgp = cpool.tile([128, 2 * T], bf, tag="sg", name="sg_t", bufs=4)[
                    :, : 2 * Wd
                ]
                for i, (wb, xg0) in enumerate(((12, 2), (17, 0))):
                    gp = psum.tile([128, T], f32, tag="psg", name="psg_t", bufs=2)[
                        :, :Wd
                    ]
                    mm_into(
                        gp,
                        [
                            (wb + 0, xt[:, xg0 * Wd : (xg0 + 1) * Wd]),
                            (wb + 1, xt[:, (xg0 + 1) * Wd : (xg0 + 2) * Wd]),
                        ],
                    )
                    nc.scalar.copy(out=sgp[:, i * Wd : (i + 1) * Wd], in_=gp)
                st["sgp"] = sgp

            def stage_a(st):
                xt, mbt, Wd, yd = st["xt"], st["mbt"], st["W"], st["yd"]

                # pvs[m,k]: x1[k] * mb[m] for m in {v0,v1,v2,s}, k in
                # {x1e_0..2, x1o_0..2} -- ONE 24W DVE op.
                pvs = pvpool.tile([128, 24 * T], bf, tag="pv", name="pv_t")[
                    :, : 24 * Wd
                ]
                nc.vector.tensor_mul(
                    pvs.rearrange("p (m k t) -> p m k t", m=4, k=6),
                    xt[:, 4 * Wd :]
                    .rearrange("p (k t) -> p k t", k=6)
                    .unsqueeze(1)
                    .broadcast_to([128, 4, 6, Wd]),
                    mbt.rearrange("p (c t) -> p c t", c=4)
                    .unsqueeze(2)
                    .broadcast_to([128, 4, 6, Wd]),
                )

                def P(m, k):  # block offset helper
                    o = (m * 6 + k) * Wd
                    return pvs[:, o : o + Wd]

                # ps0 = x0 * s (4 chunks)
                ps0 = pspool.tile([128, 4 * T], bf, tag="ps", name="ps_t")[
                    :, : 4 * Wd
                ]
                nc.vector.tensor_mul(
                    ps0.rearrange("p (c t) -> p c t", c=4),
                    xt[:, : 4 * Wd].rearrange("p (c t) -> p c t", c=4),
                    mbt[:, 3 * Wd : 4 * Wd].unsqueeze(1).broadcast_to([128, 4, Wd]),
                )

                # dots: dta = diag0 + diag1, dotp = dta + diag2 per parity
                # (a=0: 0o dot over x1e, a=1: 0e dot over x1o)
                def dpair(c):
                    # blocks {P(c,c), P(c,c+3)} -> [128, 2, Wd]
                    o = (c * 6 + c) * Wd
                    return pvs[:, o : o + 6 * Wd].rearrange(
                        "p (a k t) -> p a k t", a=2, k=3
                    )[:, :, 0, :]

                dta = cpool.tile([128, 2 * T], bf, tag="dta", name="dta_t", bufs=2)[
                    :, : 2 * Wd
                ]
                dotp = cpool.tile([128, 2 * T], bf, tag="dot", name="dot_t", bufs=2)[
                    :, : 2 * Wd
                ]
                dview = lambda ap: ap.rearrange("p (a t) -> p a t", a=2)
                nc.vector.tensor_add(dview(dta), dpair(0), dpair(1))
                nc.vector.tensor_add(dview(dotp), dview(dta), dpair(2))

                yt = ypool.tile([128, 10 * T], bf, tag="yo", name="y_t")[
                    :, : 10 * Wd
                ]
                # 0e / 0o : both m-chunks in one [2W] psum, single Act copy
                for base, wb, x0c, da in ((0, 0, 0, 1), (2, 6, 2, 0)):
                    pp = psum.tile(
                        [128, 2 * T], f32, tag="ps0", name="ps0_t", bufs=2
                    )[:, : 2 * Wd]
                    for m in range(2):
                        mm_into(
                            pp[:, m * Wd : (m + 1) * Wd],
                            [
                                (wb + 0 * 2 + m, ps0[:, x0c * Wd : (x0c + 1) * Wd]),
                                (
                                    wb + 1 * 2 + m,
                                    ps0[:, (x0c + 1) * Wd : (x0c + 2) * Wd],
                                ),
                                (wb + 2 * 2 + m, dotp[:, da * Wd : (da + 1) * Wd]),
                            ],
                        )
                    nc.scalar.copy(
                        out=yt[:, base * Wd : (base + 2) * Wd], in_=pp
                    )
                    nc.sync.dma_start(
                        out=yd[:, base * Wd : (base + 2) * Wd],
                        in_=yt[:, base * Wd : (base + 2) * Wd],
                    )
                st.update({"pvs": pvs, "yt": yt})

            def stage_b_dve(st):
                # t3[i,c] = v_c * g_i for both parities in one DVE op
                mbt, sgp, Wd = st["mbt"], st["sgp"], st["W"]
                t3p = cpool.tile([128, 6 * T], bf, tag="t3", name="t3_t", bufs=4)[
                    :, : 6 * Wd
                ]
                nc.vector.tensor_mul(
                    t3p.rearrange("p (i c t) -> p i c t", i=2, c=3),
                    mbt[:, : 3 * Wd]
                    .rearrange("p (c t) -> p c t", c=3)
                    .unsqueeze(1)
                    .broadcast_to([128, 2, 3, Wd]),
                    sgp.rearrange("p (i t) -> p i t", i=2)
                    .unsqueeze(2)
                    .broadcast_to([128, 2, 3, Wd]),
                )
                st["t3p"] = t3p

            def stage_b_pe(st):
                pvs, yt, t3p, Wd, yd = (
                    st["pvs"],
                    st["yt"],
                    st["t3p"],
                    st["W"],
                    st["yd"],
                )

                def P(m, k):
                    o = (m * 6 + k) * Wd
                    return pvs[:, o : o + Wd]

                # out1e: x1o products (k base 3), h over x1e*s (pvs m=3,k=0..2)
                # out1o: x1e products (k base 0), h over x1o*s (pvs m=3,k=3..5)
                for i, (wb, kb, hoff, ob) in enumerate(
                    ((12, 3, 18, 4), (17, 0, 21, 7))
                ):
                    # k+ : x1_a*v_b ; k- : x1_b*v_a  (a=c+1, b=c+2 mod 3)
                    # slice-major accumulation (see module docstring)
                    def contribs(c):
                        a, b = (c + 1) % 3, (c + 2) % 3
                        return [
                            (wb + 3, P(b, kb + a)),
                            (wb + 4, P(a, kb + b)),
                            (22, t3p[:, (i * 3 + c) * Wd : (i * 3 + c + 1) * Wd]),
                            (wb + 2, pvs[:, (hoff + c) * Wd : (hoff + c + 1) * Wd]),
                        ]

                    # components 0,1 share a [2W] psum + one copy; c=2 alone
                    pp = psum.tile(
                        [128, 2 * T], f32, tag="ps1", name="ps1_t", bufs=1
                    )[:, : 2 * Wd]
                    for c in range(2):
                        mm_into(pp[:, c * Wd : (c + 1) * Wd], contribs(c))
                    pc2 = psum.tile([128, T], f32, tag="psg", name="ps1c_t", bufs=2)[
                        :, :Wd
                    ]
                    mm_into(pc2, contribs(2))
                    nc.scalar.copy(out=yt[:, ob * Wd : (ob + 2) * Wd], in_=pp)
                    nc.scalar.copy(
                        out=yt[:, (ob + 2) * Wd : (ob + 3) * Wd], in_=pc2
                    )
                    nc.sync.dma_start(
                        out=yd[:, ob * Wd : (ob + 3) * Wd],
                        in_=yt[:, ob * Wd : (ob + 3) * Wd],
                    )

            # software pipeline: loads prefetched one seg ahead, stage B
            # (t3 + 1e/1o matmuls + store) one seg behind stage A
            states = {0: load(segs[0])}
            # weights load queued after seg 0's data so the DVE-critical
            # descriptors go out first (PE touches weights later anyway)
            nc.sync.dma_start(out=wt[:, :], in_=w[:, :])
            for i in range(nseg):
                if i + 1 < nseg:
                    states[i + 1] = load(segs[i + 1])
                stage_g(states[i])
                if i >= 1:
                    stage_b_dve(states[i - 1])
                    stage_b_pe(states[i - 1])
                stage_a(states[i])
                if i >= 1:
                    del states[i - 1]
            stage_b_dve(states[nseg - 1])
            stage_b_pe(states[nseg - 1])
    nc.finalize()
    return nc


_PROG_CACHE = {}


def _get_program(Bs):
    if Bs not in _PROG_CACHE:
        _PROG_CACHE[Bs] = _build_program(Bs)
    return _PROG_CACHE[Bs]


def run(inputs, trace=False, **kw):
    in1 = np.asarray(inputs["in1"], np.float32)
    in2 = np.asarray(inputs["in2"], np.float32)
    B = in1.shape[0]
    assert B % (N_CORES * T) == 0, B
    Bs = B // N_CORES

    wpk = _pack_weights(
        np.asarray(inputs["W0e"], np.float32),
        np.asarray(inputs["W0o"], np.float32),
        np.asarray(inputs["W1e"], np.float32),
        np.asarray(inputs["W1o"], np.float32),
    )

    in_maps = []
    for i in range(N_CORES):
        ssl = slice(i * Bs, (i + 1) * Bs)
        xm, s4m, xew, s4ew = _prep_shard(in1[ssl], in2[ssl])
        in_maps.append({"x": xm, "s4": s4m, "xe": xew, "s4e": s4ew, "w": wpk})

    nc = _get_program(Bs)
    res = run_bass_kernel_spmd(nc, in_maps, list(range(N_CORES)), trace=trace, **kw)

    out = np.empty((B, 1280), np.float32)
    for i in range(N_CORES):
        out[i * Bs : (i + 1) * Bs] = _post_shard(
            res.results[i]["y"], res.results[i]["ye"], Bs
        )
    return out, res


def kernel(**inputs):
    out, _ = run(inputs, trace=False)
    return out


# revision 14
# speedup vs baseline: 1.0526x; 1.0526x over previous
"""Trainium2 Bass kernel for the L1 tensor-product problem.

Math (per batch row b):
  out0e = [x0e*s, CG*(x1o.v)] @ W0e * NORM0E
  out0o = [x0o*s, CG*(x1e.v)] @ W0o * NORM0O
  out1e_c = [CG*x0o*v_c, CG*x1e_c*s, CGC*cross(x1o,v)_c] @ W1e * NORM1E
  out1o_c = [CG*x0e*v_c, CG*x1o_c*s, CGC*cross(x1e,v)_c] @ W1o * NORM1O

Kernel strategy (pure data parallel over batch, 8 cores), v5:
  * bf16 wire + matmul dtype; PSUM accumulates fp32 (rel-err ~4e-3,
    budget 2e-2).
  * DVE is the bottleneck engine (~100% busy): all per-row products run
    as 2x-mode bf16 tensor_tensor ops at the hw max of ~1.92 elem/ns,
    and the schedule keeps DVE *elements* at the formulation's minimum
    (38 chunks per tile-column):
      - pvs: ONE 24T op computes x1 x {v0, v1, v2, s} in an [m,k,t]
        layout: x1e*s / x1o*s land contiguous for the h-path matmuls
        and diag/off-diag blocks at AP-addressable offsets for the
        k+/k- matmuls and the dots.
      - dots (2 adds, paired across parities), t3 = v_c*g (6T).
  * Unscaled g = x0?' @ Wg at the head of each PE seg so sgp is ready
    a full seg before the t3 op that consumes it.
  * First/last tile are split into 4 W=128 subtiles so the pipeline
    ramp (first load -> first DVE op) and the tail drain chain are a
    quarter-length.  Edge wedges are packed CONTIGUOUSLY on the host
    (xe/ye/s4e params) -- slicing wedges out of the tile-major layout
    shatters the DMA into 256B packets (measured 40k packets, DMA 86%
    busy, DVE starved at 80%).
  * All DMA in/out APs are 2D contiguous slices; each partition's
    tile-load is one 10KB descriptor run.
  * Multiplier rows (v,s) reach all 128 partitions via a stride-0
    broadcast DMA read (GpSimd is unusable: its SBUF port contends
    with 2-port DVE ops and its tensor ops trip the chip's utilization
    throttle; PE ones-broadcasts would eat the PE headroom).
  * PSUM accumulation is slice-major (see memory: interleaving
    start/stop groups across slices of one psum region is wrong on hw).
"""

import sys

sys.path.insert(0, "/opt/trn_rl_repo")

import numpy as np

import concourse.bass as bass
import concourse.bacc as bacc
import concourse.mybir as mybir
from concourse.bass_utils import run_bass_kernel_spmd
from concourse.tile import TileContext

N_CORES = 8
T = 512  # batch columns per full tile
WEDGE = 128  # subtile width for first/last tile
NW = T // WEDGE  # wedges per edge tile

# irreps: 256x0e + 256x0o + 128x1e + 128x1o
CG = 1.0 / 3.0**0.5
CGC = 1.0 / 6.0**0.5
NORM0E = (1.0 / 384.0) ** 0.5
NORM0O = (1.0 / 384.0) ** 0.5
NORM1E = (3.0 / 512.0) ** 0.5
NORM1O = (3.0 / 512.0) ** 0.5

_BF16 = None


def _bf16():
    global _BF16
    if _BF16 is None:
        import ml_dtypes

        _BF16 = np.dtype(ml_dtypes.bfloat16)
    return _BF16


def _pack_weights(W0e, W0o, W1e, W1o):
    """Fold constants/signs; 22 lhsT chunks [128,128] side by side.

    Order: 0e (kc0m0,kc0m1,kc1m0,kc1m1,kc2m0,kc2m1), 0o (same 6),
    1e (g0,g1,h,k+,k-), 1o (g0,g1,h,k+,k-), identity.
    """
    W0e = W0e.astype(np.float64) * NORM0E
    W0e[256:] *= CG
    W0o = W0o.astype(np.float64) * NORM0O
    W0o[256:] *= CG
    W1e = W1e.astype(np.float64) * NORM1E
    W1e[:384] *= CG
    W1e[384:] *= CGC
    W1o = W1o.astype(np.float64) * NORM1O
    W1o[:384] *= CG
    W1o[384:] *= CGC
    chunks = []
    for W in (W0e, W0o):  # [384, 256]
        for kc in range(3):
            for mc in range(2):
                chunks.append(W[kc * 128 : (kc + 1) * 128, mc * 128 : (mc + 1) * 128])
    for W in (W1e, W1o):  # [512, 128]
        chunks.append(W[0:128, :])      # g0
        chunks.append(W[128:256, :])    # g1
        chunks.append(W[256:384, :])    # h
        chunks.append(W[384:512, :])    # k+
        chunks.append(-W[384:512, :])   # k-
    chunks.append(np.eye(128, dtype=np.float64))  # 22: identity (combine accum)
    packed = np.concatenate(chunks, axis=1)
    return np.ascontiguousarray(packed.astype(_bf16()))


def _pack_rows(in1_r, in2_r, Wd):
    """Pack a block of Wd rows: -> x [128, 10*Wd] bf16, s4 [4, Wd] bf16.

    Chunk order: 0,1=x0e  2,3=x0o  4+c=x1e_c  7+c=x1o_c.
    Multiplier rows in [v0, v1, v2, s] order.
    """
    dt = _bf16()
    x = np.empty((128, 10, Wd), dt)
    x[:, 0:4] = in1_r[:, 0:512].reshape(Wd, 4, 128).transpose(2, 1, 0)
    x[:, 4:7] = in1_r[:, 512:896].reshape(Wd, 128, 3).transpose(1, 2, 0)
    x[:, 7:10] = in1_r[:, 896:1280].reshape(Wd, 128, 3).transpose(1, 2, 0)
    s4 = np.ascontiguousarray(in2_r[:, [1, 2, 3, 0]].T.astype(dt))
    return np.ascontiguousarray(x.reshape(128, 10 * Wd)), s4


def _prep_shard(in1_s, in2_s):
    """Middle tiles tile-major + edge wedges contiguous."""
    Bs = in1_s.shape[0]
    nt = Bs // T
    dt = _bf16()
    if nt == 1:
        nmid = 0
        edges = [(j * WEDGE, WEDGE) for j in range(NW)]
    else:
        nmid = nt - 2
        edges = [(j * WEDGE, WEDGE) for j in range(NW)] + [
            (Bs - T + j * WEDGE, WEDGE) for j in range(NW)
        ]
    xm = np.empty((max(nmid, 1), 128, 10 * T), dt)
    s4m = np.empty((max(nmid, 1), 4, T), dt)
    for t in range(nmid):
        r = slice(T + t * T, T + (t + 1) * T)
        xm[t], s4m[t] = _pack_rows(in1_s[r], in2_s[r], T)
    ne = len(edges)
    xe = np.empty((ne, 128, 10 * WEDGE), dt)
    s4e = np.empty((ne, 4, WEDGE), dt)
    for j, (off, Wd) in enumerate(edges):
        xe[j], s4e[j] = _pack_rows(in1_s[off : off + Wd], in2_s[off : off + Wd], Wd)
    return xm, s4m, xe, s4e


def _unpack_block(yb, Wd):
    """[128, 10*Wd] bf16 -> [Wd, 1280] fp32."""
    yb = np.asarray(yb).reshape(128, 10, Wd).astype(np.float32)
    out = np.empty((Wd, 1280), np.float32)
    out[:, 0:512] = yb[:, 0:4].transpose(2, 1, 0).reshape(Wd, 512)
    out[:, 512:896] = yb[:, 4:7].transpose(2, 0, 1).reshape(Wd, 384)
    out[:, 896:1280] = yb[:, 7:10].transpose(2, 0, 1).reshape(Wd, 384)
    return out


def _post_shard(ym, ye, Bs):
    nt = Bs // T
    out = np.empty((Bs, 1280), np.float32)
    if nt == 1:
        for j in range(NW):
            out[j * WEDGE : (j + 1) * WEDGE] = _unpack_block(ye[j], WEDGE)
        return out
    for t in range(nt - 2):
        out[T + t * T : T + (t + 1) * T] = _unpack_block(ym[t], T)
    for j in range(NW):
        out[j * WEDGE : (j + 1) * WEDGE] = _unpack_block(ye[j], WEDGE)
        out[Bs - T + j * WEDGE : Bs - T + (j + 1) * WEDGE] = _unpack_block(
            ye[NW + j], WEDGE
        )
    return out


def _build_program(Bs):
    assert Bs % T == 0, (Bs, T)
    nt = Bs // T
    nmid = 0 if nt == 1 else nt - 2
    ne = NW if nt == 1 else 2 * NW
    bf = mybir.dt.bfloat16
    f32 = mybir.dt.float32

    nc = bacc.Bacc()
    x = nc.declare_dram_parameter("x", [max(nmid, 1), 128, 10 * T], bf, isOutput=False)
    s4 = nc.declare_dram_parameter("s4", [max(nmid, 1), 4, T], bf, isOutput=False)
    xe = nc.declare_dram_parameter("xe", [ne, 128, 10 * WEDGE], bf, isOutput=False)
    s4e = nc.declare_dram_parameter("s4e", [ne, 4, WEDGE], bf, isOutput=False)
    w = nc.declare_dram_parameter("w", [128, 23 * 128], bf, isOutput=False)
    y = nc.declare_dram_parameter("y", [max(nmid, 1), 128, 10 * T], bf, isOutput=True)
    ye = nc.declare_dram_parameter("ye", [ne, 128, 10 * WEDGE], bf, isOutput=True)

    # segment order: leading wedges, middle tiles, trailing wedges
    segs = [("e", j, WEDGE) for j in range(NW)]
    segs += [("m", t, T) for t in range(nmid)]
    if nt > 1:
        segs += [("e", NW + j, WEDGE) for j in range(NW)]
    nseg = len(segs)

    with TileContext(nc) as tc:
        with (
            tc.tile_pool(name="wpool", bufs=1) as wpool,
            tc.tile_pool(name="xpool", bufs=4) as xpool,
            tc.tile_pool(name="mbpool", bufs=4) as mbpool,
            tc.tile_pool(name="pvpool", bufs=2) as pvpool,
            tc.tile_pool(name="pspool", bufs=3) as pspool,
            tc.tile_pool(name="cpool", bufs=2) as cpool,
            tc.tile_pool(name="ypool", bufs=2) as ypool,
            tc.tile_pool(name="psum", bufs=8, space="PSUM") as psum,
        ):
            wt = wpool.tile([128, 23 * 128], bf)

            def W(i):
                return wt[:, i * 128 : (i + 1) * 128]

            def load_wedge_block(j0):
                """ONE set of dma_starts for NW wedges (the per-seg Sync
                issue cost of ~0.6-1.4us per dma_start otherwise starves
                the short wedge segs)."""
                WB = 10 * WEDGE
                xt4 = xpool.tile([128, 10 * T], bf, tag="xt", name="x_t")
                xv = xe[j0 : j0 + NW].rearrange("j p c -> p j c")
                ov = xt4[:, : NW * WB].rearrange("p (j c) -> p j c", j=NW)
                nc.sync.dma_start(out=ov[:, :, 4 * WEDGE :], in_=xv[:, :, 4 * WEDGE :])
                nc.sync.dma_start(out=ov[:, :, : 4 * WEDGE], in_=xv[:, :, : 4 * WEDGE])
                mbt4 = mbpool.tile([128, 4 * T], bf, tag="mb", name="mb_t")
                nc.sync.dma_start(
                    out=mbt4[:, : NW * 4 * WEDGE].rearrange(
                        "p (j c t) -> p j c t", j=NW, c=4
                    ),
                    in_=s4e[j0 : j0 + NW]
                    .unsqueeze(0)
                    .broadcast_to([128, NW, 4, WEDGE]),
                )
                out = {}
                for j in range(NW):
                    out[j] = {
                        "xt": xt4[:, j * WB : (j + 1) * WB],
                        "mbt": mbt4[:, j * 4 * WEDGE : (j + 1) * 4 * WEDGE],
                        "W": WEDGE,
                        "yd": ye[j0 + j],
                        "wedge": True,
                    }
                return out

            def load_mid(t):
                xt = xpool.tile([128, 10 * T], bf, tag="xt", name="x_t")
                nc.sync.dma_start(out=xt[:, :], in_=x[t])
                mbt = mbpool.tile([128, 4 * T], bf, tag="mb", name="mb_t")
                nc.sync.dma_start(
                    out=mbt[:, :].rearrange("p (c t) -> p c t", c=4),
                    in_=s4[t].unsqueeze(0).broadcast_to([128, 4, T]),
                )
                return {"xt": xt, "mbt": mbt, "W": T, "yd": y[t], "wedge": False}

            def mm_into(p, contribs, first=True, last=True):
                n = len(contribs)
                for i, (wi, rhs) in enumerate(contribs):
                    nc.tensor.matmul(
                        p,
                        W(wi),
                        rhs,
                        start=(first and i == 0),
                        stop=(last and i == n - 1),
                    )

            def stage_g(st):
                # g = x0?' @ Wg (unscaled; only needs xt).  Emitted at the
                # head of each PE seg so sgp is ready a full seg before the
                # t3 op that consumes it.
                xt, Wd = st["xt"], st["W"]
                sgp = cpool.tile([128, 2 * T], bf, tag="sg", name="sg_t", bufs=4)[
                    :, : 2 * Wd
                ]
                for i, (wb, xg0) in enumerate(((12, 2), (17, 0))):
                    gp = psum.tile([128, T], f32, tag="psg", name="psg_t", bufs=2)[
                        :, :Wd
                    ]
                    mm_into(
                        gp,
                        [
                            (wb + 0, xt[:, xg0 * Wd : (xg0 + 1) * Wd]),
                            (wb + 1, xt[:, (xg0 + 1) * Wd : (xg0 + 2) * Wd]),
                        ],
                    )
                    nc.scalar.copy(out=sgp[:, i * Wd : (i + 1) * Wd], in_=gp)
                st["sgp"] = sgp

            def stage_a(st):
                xt, mbt, Wd, yd = st["xt"], st["mbt"], st["W"], st["yd"]

                # pvs[m,k]: x1[k] * mb[m] for m in {v0,v1,v2,s}, k in
                # {x1e_0..2, x1o_0..2} -- ONE 24W DVE op.
                pvs = pvpool.tile([128, 24 * T], bf, tag="pv", name="pv_t")[
                    :, : 24 * Wd
                ]
                nc.vector.tensor_mul(
                    pvs.rearrange("p (m k t) -> p m k t", m=4, k=6),
                    xt[:, 4 * Wd :]
                    .rearrange("p (k t) -> p k t", k=6)
                    .unsqueeze(1)
                    .broadcast_to([128, 4, 6, Wd]),
                    mbt.rearrange("p (c t) -> p c t", c=4)
                    .unsqueeze(2)
                    .broadcast_to([128, 4, 6, Wd]),
                )

                def P(m, k):  # block offset helper
                    o = (m * 6 + k) * Wd
                    return pvs[:, o : o + Wd]

                # ps0 = x0 * s (4 chunks)
                ps0 = pspool.tile([128, 4 * T], bf, tag="ps", name="ps_t")[
                    :, : 4 * Wd
                ]
                nc.vector.tensor_mul(
                    ps0.rearrange("p (c t) -> p c t", c=4),
                    xt[:, : 4 * Wd].rearrange("p (c t) -> p c t", c=4),
                    mbt[:, 3 * Wd : 4 * Wd].unsqueeze(1).broadcast_to([128, 4, Wd]),
                )

                # dots: dta = diag0 + diag1, dotp = dta + diag2 per parity
                # (a=0: 0o dot over x1e, a=1: 0e dot over x1o)
                def dpair(c):
                    # blocks {P(c,c), P(c,c+3)} -> [128, 2, Wd]
                    o = (c * 6 + c) * Wd
                    return pvs[:, o : o + 6 * Wd].rearrange(
                        "p (a k t) -> p a k t", a=2, k=3
                    )[:, :, 0, :]

                dta = cpool.tile([128, 2 * T], bf, tag="dta", name="dta_t", bufs=2)[
                    :, : 2 * Wd
                ]
                dotp = cpool.tile([128, 2 * T], bf, tag="dot", name="dot_t", bufs=2)[
                    :, : 2 * Wd
                ]
                dview = lambda ap: ap.rearrange("p (a t) -> p a t", a=2)
                nc.vector.tensor_add(dview(dta), dpair(0), dpair(1))
                nc.vector.tensor_add(dview(dotp), dview(dta), dpair(2))

                yt = ypool.tile([128, 10 * T], bf, tag="yo", name="y_t")[
                    :, : 10 * Wd
                ]
                # 0e / 0o : both m-chunks in one [2W] psum, single Act copy
                for base, wb, x0c, da in ((0, 0, 0, 1), (2, 6, 2, 0)):
                    pp = psum.tile(
                        [128, 2 * T], f32, tag="ps0", name="ps0_t", bufs=2
                    )[:, : 2 * Wd]
                    for m in range(2):
                        mm_into(
                            pp[:, m * Wd : (m + 1) * Wd],
                            [
                                (wb + 0 * 2 + m, ps0[:, x0c * Wd : (x0c + 1) * Wd]),
                                (
                                    wb + 1 * 2 + m,
                                    ps0[:, (x0c + 1) * Wd : (x0c + 2) * Wd],
                                ),
                                (wb + 2 * 2 + m, dotp[:, da * Wd : (da + 1) * Wd]),
                            ],
                        )
                    nc.scalar.copy(
                        out=yt[:, base * Wd : (base + 2) * Wd], in_=pp
                    )
                if not st["wedge"]:
                    # store 0e+0o as one transfer (wedges store once at
                    # the end of stage_b to minimize Sync issues)
                    nc.sync.dma_start(
                        out=yd[:, : 4 * Wd], in_=yt[:, : 4 * Wd]
                    )
                st.update({"pvs": pvs, "yt": yt})

            def stage_b_dve(st):
                # t3[i,c] = v_c * g_i for both parities in one DVE op
                mbt, sgp, Wd = st["mbt"], st["sgp"], st["W"]
                t3p = cpool.tile([128, 6 * T], bf, tag="t3", name="t3_t", bufs=4)[
                    :, : 6 * Wd
                ]
                nc.vector.tensor_mul(
                    t3p.rearrange("p (i c t) -> p i c t", i=2, c=3),
                    mbt[:, : 3 * Wd]
                    .rearrange("p (c t) -> p c t", c=3)
                    .unsqueeze(1)
                    .broadcast_to([128, 2, 3, Wd]),
                    sgp.rearrange("p (i t) -> p i t", i=2)
                    .unsqueeze(2)
                    .broadcast_to([128, 2, 3, Wd]),
                )
                st["t3p"] = t3p

            def stage_b_pe(st):
                pvs, yt, t3p, Wd, yd = (
                    st["pvs"],
                    st["yt"],
                    st["t3p"],
                    st["W"],
                    st["yd"],
                )

                def P(m, k):
                    o = (m * 6 + k) * Wd
                    return pvs[:, o : o + Wd]

                # out1e: x1o products (k base 3), h over x1e*s (pvs m=3,k=0..2)
                # out1o: x1e products (k base 0), h over x1o*s (pvs m=3,k=3..5)
                for i, (wb, kb, hoff, ob) in enumerate(
                    ((12, 3, 18, 4), (17, 0, 21, 7))
                ):
                    # k+ : x1_a*v_b ; k- : x1_b*v_a  (a=c+1, b=c+2 mod 3)
                    # slice-major accumulation (see module docstring)
                    def contribs(c):
                        a, b = (c + 1) % 3, (c + 2) % 3
                        return [
                            (wb + 3, P(b, kb + a)),
                            (wb + 4, P(a, kb + b)),
                            (22, t3p[:, (i * 3 + c) * Wd : (i * 3 + c + 1) * Wd]),
                            (wb + 2, pvs[:, (hoff + c) * Wd : (hoff + c + 1) * Wd]),
                        ]

                    # components 0,1 share a [2W] psum + one copy; c=2 alone
                    pp = psum.tile(
                        [128, 2 * T], f32, tag="ps1", name="ps1_t", bufs=1
                    )[:, : 2 * Wd]
                    for c in range(2):
                        mm_into(pp[:, c * Wd : (c + 1) * Wd], contribs(c))
                    pc2 = psum.tile([128, T], f32, tag="psg", name="ps1c_t", bufs=2)[
                        :, :Wd
                    ]
                    mm_into(pc2, contribs(2))
                    nc.scalar.copy(out=yt[:, ob * Wd : (ob + 2) * Wd], in_=pp)
                    nc.scalar.copy(
                        out=yt[:, (ob + 2) * Wd : (ob + 3) * Wd], in_=pc2
                    )
                if st["wedge"]:
                    nc.sync.dma_start(out=yd[:, :], in_=yt)
                else:
                    nc.sync.dma_start(
                        out=yd[:, 4 * Wd :], in_=yt[:, 4 * Wd :]
                    )

            # software pipeline: loads prefetched TWO segs ahead (a full
            # tile load is ~4.5us; a wedge seg is only ~3us of DVE work),
            # stage B (t3 + 1e/1o matmuls + store) one seg behind stage A
            states = load_wedge_block(0)
            nc.sync.dma_start(out=wt[:, :], in_=w[:, :])
            trail_loaded = nt == 1
            for i in range(nseg):
                j = i + 2
                if j < nseg and j not in states:
                    kind, t, _ = segs[j]
                    if kind == "m":
                        states[j] = load_mid(t)
                    elif not trail_loaded:
                        states.update(
                            {NW + nmid + jj: st
                             for jj, st in load_wedge_block(NW).items()}
                        )
                        trail_loaded = True
                stage_g(states[i])
                if i >= 1:
                    stage_b_dve(states[i - 1])
                    stage_b_pe(states[i - 1])
                stage_a(states[i])
                if i >= 1:
                    del states[i - 1]
            stage_b_dve(states[nseg - 1])
            stage_b_pe(states[nseg - 1])
    nc.finalize()
    return nc


_PROG_CACHE = {}


def _get_program(Bs):
    if Bs not in _PROG_CACHE:
        _PROG_CACHE[Bs] = _build_program(Bs)
    return _PROG_CACHE[Bs]


def run(inputs, trace=False, **kw):
    in1 = np.asarray(inputs["in1"], np.float32)
    in2 = np.asarray(inputs["in2"], np.float32)
    B = in1.shape[0]
    assert B % (N_CORES * T) == 0, B
    Bs = B // N_CORES

    wpk = _pack_weights(
        np.asarray(inputs["W0e"], np.float32),
        np.asarray(inputs["W0o"], np.float32),
        np.asarray(inputs["W1e"], np.float32),
        np.asarray(inputs["W1o"], np.float32),
    )

    in_maps = []
    for i in range(N_CORES):
        ssl = slice(i * Bs, (i + 1) * Bs)
        xm, s4m, xew, s4ew = _prep_shard(in1[ssl], in2[ssl])
        in_maps.append({"x": xm, "s4": s4m, "xe": xew, "s4e": s4ew, "w": wpk})

    nc = _get_program(Bs)
    res = run_bass_kernel_spmd(nc, in_maps, list(range(N_CORES)), trace=trace, **kw)

    out = np.empty((B, 1280), np.float32)
    for i in range(N_CORES):
        out[i * Bs : (i + 1) * Bs] = _post_shard(
            res.results[i]["y"], res.results[i]["ye"], Bs
        )
    return out, res


def kernel(**inputs):
    out, _ = run(inputs, trace=False)
    return out


# revision 16
# speedup vs baseline: 1.0913x; 1.0367x over previous
"""Trainium2 Bass kernel for the L1 tensor-product problem.

Math (per batch row b):
  out0e = [x0e*s, CG*(x1o.v)] @ W0e * NORM0E
  out0o = [x0o*s, CG*(x1e.v)] @ W0o * NORM0O
  out1e_c = [CG*x0o*v_c, CG*x1e_c*s, CGC*cross(x1o,v)_c] @ W1e * NORM1E
  out1o_c = [CG*x0e*v_c, CG*x1o_c*s, CGC*cross(x1e,v)_c] @ W1o * NORM1O

Kernel strategy (pure data parallel over batch, 8 cores), v5:
  * bf16 wire + matmul dtype; PSUM accumulates fp32 (rel-err ~4e-3,
    budget 2e-2).
  * DVE is the bottleneck engine (~100% busy): all per-row products run
    as 2x-mode bf16 tensor_tensor ops at the hw max of ~1.92 elem/ns,
    and the schedule keeps DVE *elements* at the formulation's minimum
    (38 chunks per tile-column):
      - pvs: ONE 24T op computes x1 x {v0, v1, v2, s} in an [m,k,t]
        layout: x1e*s / x1o*s land contiguous for the h-path matmuls
        and diag/off-diag blocks at AP-addressable offsets for the
        k+/k- matmuls and the dots.
      - dots (2 adds, paired across parities), t3 = v_c*g (6T).
  * Unscaled g = x0?' @ Wg at the head of each PE seg so sgp is ready
    a full seg before the t3 op that consumes it.
  * First/last tile are split into 4 W=128 subtiles so the pipeline
    ramp (first load -> first DVE op) and the tail drain chain are a
    quarter-length.  Edge wedges are packed CONTIGUOUSLY on the host
    (xe/ye/s4e params) -- slicing wedges out of the tile-major layout
    shatters the DMA into 256B packets (measured 40k packets, DMA 86%
    busy, DVE starved at 80%).
  * All DMA in/out APs are 2D contiguous slices; each partition's
    tile-load is one 10KB descriptor run.
  * Multiplier rows (v,s) reach all 128 partitions via a stride-0
    broadcast DMA read (GpSimd is unusable: its SBUF port contends
    with 2-port DVE ops and its tensor ops trip the chip's utilization
    throttle; PE ones-broadcasts would eat the PE headroom).
  * PSUM accumulation is slice-major (see memory: interleaving
    start/stop groups across slices of one psum region is wrong on hw).
"""

import sys

sys.path.insert(0, "/opt/trn_rl_repo")

import numpy as np

import concourse.bass as bass
import concourse.bacc as bacc
import concourse.mybir as mybir
from concourse.bass_utils import run_bass_kernel_spmd
from concourse.tile import TileContext

N_CORES = 8
T = 512  # batch columns per full tile
WEDGE = 256  # subtile width for the last (drain) tile
NW = T // WEDGE  # wedges in the drain tile

# irreps: 256x0e + 256x0o + 128x1e + 128x1o
CG = 1.0 / 3.0**0.5
CGC = 1.0 / 6.0**0.5
NORM0E = (1.0 / 384.0) ** 0.5
NORM0O = (1.0 / 384.0) ** 0.5
NORM1E = (3.0 / 512.0) ** 0.5
NORM1O = (3.0 / 512.0) ** 0.5

_BF16 = None


def _bf16():
    global _BF16
    if _BF16 is None:
        import ml_dtypes

        _BF16 = np.dtype(ml_dtypes.bfloat16)
    return _BF16


def _pack_weights(W0e, W0o, W1e, W1o):
    """Fold constants/signs; 22 lhsT chunks [128,128] side by side.

    Order: 0e (kc0m0,kc0m1,kc1m0,kc1m1,kc2m0,kc2m1), 0o (same 6),
    1e (g0,g1,h,k+,k-), 1o (g0,g1,h,k+,k-), identity.
    """
    W0e = W0e.astype(np.float64) * NORM0E
    W0e[256:] *= CG
    W0o = W0o.astype(np.float64) * NORM0O
    W0o[256:] *= CG
    W1e = W1e.astype(np.float64) * NORM1E
    W1e[:384] *= CG
    W1e[384:] *= CGC
    W1o = W1o.astype(np.float64) * NORM1O
    W1o[:384] *= CG
    W1o[384:] *= CGC
    chunks = []
    for W in (W0e, W0o):  # [384, 256]
        for kc in range(3):
            for mc in range(2):
                chunks.append(W[kc * 128 : (kc + 1) * 128, mc * 128 : (mc + 1) * 128])
    for W in (W1e, W1o):  # [512, 128]
        chunks.append(W[0:128, :])      # g0
        chunks.append(W[128:256, :])    # g1
        chunks.append(W[256:384, :])    # h
        chunks.append(W[384:512, :])    # k+
        chunks.append(-W[384:512, :])   # k-
    chunks.append(np.eye(128, dtype=np.float64))  # 22: identity (combine accum)
    packed = np.concatenate(chunks, axis=1)
    return np.ascontiguousarray(packed.astype(_bf16()))


def _pack_rows(in1_r, in2_r, Wd):
    """Pack a block of Wd rows: -> x [128, 10*Wd] bf16, s4 [4, Wd] bf16.

    Chunk order: 0,1=x0e  2,3=x0o  4+c=x1e_c  7+c=x1o_c.
    Multiplier rows in [v0, v1, v2, s] order.
    """
    dt = _bf16()
    x = np.empty((128, 10, Wd), dt)
    x[:, 0:4] = in1_r[:, 0:512].reshape(Wd, 4, 128).transpose(2, 1, 0)
    x[:, 4:7] = in1_r[:, 512:896].reshape(Wd, 128, 3).transpose(1, 2, 0)
    x[:, 7:10] = in1_r[:, 896:1280].reshape(Wd, 128, 3).transpose(1, 2, 0)
    s4 = np.ascontiguousarray(in2_r[:, [1, 2, 3, 0]].T.astype(dt))
    return np.ascontiguousarray(x.reshape(128, 10 * Wd)), s4


def _prep_shard(in1_s, in2_s):
    """Leading tiles tile-major + trailing (drain) wedges contiguous."""
    Bs = in1_s.shape[0]
    nt = Bs // T
    dt = _bf16()
    nmid = nt - 1 if nt > 1 else 0
    edges = [(Bs - T + j * WEDGE, WEDGE) for j in range(NW)]
    xm = np.empty((max(nmid, 1), 128, 10 * T), dt)
    s4m = np.empty((max(nmid, 1), 4, T), dt)
    for t in range(nmid):
        r = slice(t * T, (t + 1) * T)
        xm[t], s4m[t] = _pack_rows(in1_s[r], in2_s[r], T)
    ne = len(edges)
    xe = np.empty((ne, 128, 10 * WEDGE), dt)
    s4e = np.empty((ne, 4, WEDGE), dt)
    for j, (off, Wd) in enumerate(edges):
        xe[j], s4e[j] = _pack_rows(in1_s[off : off + Wd], in2_s[off : off + Wd], Wd)
    return xm, s4m, xe, s4e


def _unpack_block(yb, Wd):
    """[128, 10*Wd] bf16 -> [Wd, 1280] fp32."""
    yb = np.asarray(yb).reshape(128, 10, Wd).astype(np.float32)
    out = np.empty((Wd, 1280), np.float32)
    out[:, 0:512] = yb[:, 0:4].transpose(2, 1, 0).reshape(Wd, 512)
    out[:, 512:896] = yb[:, 4:7].transpose(2, 0, 1).reshape(Wd, 384)
    out[:, 896:1280] = yb[:, 7:10].transpose(2, 0, 1).reshape(Wd, 384)
    return out


def _post_shard(ym, ye, Bs):
    nt = Bs // T
    out = np.empty((Bs, 1280), np.float32)
    nmid = nt - 1 if nt > 1 else 0
    for t in range(nmid):
        out[t * T : (t + 1) * T] = _unpack_block(ym[t], T)
    for j in range(NW):
        out[Bs - T + j * WEDGE : Bs - T + (j + 1) * WEDGE] = _unpack_block(
            ye[j], WEDGE
        )
    return out


def _build_program(Bs):
    assert Bs % T == 0, (Bs, T)
    nt = Bs // T
    nmid = nt - 1 if nt > 1 else 0
    ne = NW
    bf = mybir.dt.bfloat16
    f32 = mybir.dt.float32

    nc = bacc.Bacc()
    x = nc.declare_dram_parameter("x", [max(nmid, 1), 128, 10 * T], bf, isOutput=False)
    s4 = nc.declare_dram_parameter("s4", [max(nmid, 1), 4, T], bf, isOutput=False)
    xe = nc.declare_dram_parameter("xe", [ne, 128, 10 * WEDGE], bf, isOutput=False)
    s4e = nc.declare_dram_parameter("s4e", [ne, 4, WEDGE], bf, isOutput=False)
    w = nc.declare_dram_parameter("w", [128, 23 * 128], bf, isOutput=False)
    y = nc.declare_dram_parameter("y", [max(nmid, 1), 128, 10 * T], bf, isOutput=True)
    ye = nc.declare_dram_parameter("ye", [ne, 128, 10 * WEDGE], bf, isOutput=True)

    # segment order: full tiles, then drain wedges
    segs = [("m", t, T) for t in range(nmid)]
    segs += [("e", j, WEDGE) for j in range(NW)]
    nseg = len(segs)

    with TileContext(nc) as tc:
        with (
            tc.tile_pool(name="wpool", bufs=1) as wpool,
            tc.tile_pool(name="xpool", bufs=4) as xpool,
            tc.tile_pool(name="mbpool", bufs=4) as mbpool,
            tc.tile_pool(name="pvpool", bufs=2) as pvpool,
            tc.tile_pool(name="pspool", bufs=3) as pspool,
            tc.tile_pool(name="cpool", bufs=2) as cpool,
            tc.tile_pool(name="ypool", bufs=2) as ypool,
            tc.tile_pool(name="psum", bufs=8, space="PSUM") as psum,
        ):
            wt = wpool.tile([128, 23 * 128], bf)

            def W(i):
                return wt[:, i * 128 : (i + 1) * 128]

            def load_wedge_block(j0):
                """ONE set of dma_starts for NW wedges (the per-seg Sync
                issue cost of ~0.6-1.4us per dma_start otherwise starves
                the short wedge segs)."""
                WB = 10 * WEDGE
                xt4 = xpool.tile([128, 10 * T], bf, tag="xt", name="x_t")
                xv = xe[j0 : j0 + NW].rearrange("j p c -> p j c")
                ov = xt4[:, : NW * WB].rearrange("p (j c) -> p j c", j=NW)
                nc.sync.dma_start(out=ov[:, :, 4 * WEDGE :], in_=xv[:, :, 4 * WEDGE :])
                nc.sync.dma_start(out=ov[:, :, : 4 * WEDGE], in_=xv[:, :, : 4 * WEDGE])
                mbt4 = mbpool.tile([128, 4 * T], bf, tag="mb", name="mb_t")
                nc.sync.dma_start(
                    out=mbt4[:, : NW * 4 * WEDGE].rearrange(
                        "p (j c t) -> p j c t", j=NW, c=4
                    ),
                    in_=s4e[j0 : j0 + NW]
                    .unsqueeze(0)
                    .broadcast_to([128, NW, 4, WEDGE]),
                )
                out = {}
                for j in range(NW):
                    out[j] = {
                        "xt": xt4[:, j * WB : (j + 1) * WB],
                        "mbt": mbt4[:, j * 4 * WEDGE : (j + 1) * 4 * WEDGE],
                        "W": WEDGE,
                        "yd": ye[j0 + j],
                        "wedge": True,
                    }
                return out

            def load_mid(t):
                xt = xpool.tile([128, 10 * T], bf, tag="xt", name="x_t")
                nc.sync.dma_start(out=xt[:, :], in_=x[t])
                mbt = mbpool.tile([128, 4 * T], bf, tag="mb", name="mb_t")
                nc.sync.dma_start(
                    out=mbt[:, :].rearrange("p (c t) -> p c t", c=4),
                    in_=s4[t].unsqueeze(0).broadcast_to([128, 4, T]),
                )
                return {"xt": xt, "mbt": mbt, "W": T, "yd": y[t], "wedge": False}

            def mm_into(p, contribs, first=True, last=True):
                n = len(contribs)
                for i, (wi, rhs) in enumerate(contribs):
                    nc.tensor.matmul(
                        p,
                        W(wi),
                        rhs,
                        start=(first and i == 0),
                        stop=(last and i == n - 1),
                    )

            def stage_g(st):
                # g = x0?' @ Wg (unscaled; only needs xt).  Emitted at the
                # head of each PE seg so sgp is ready a full seg before the
                # t3 op that consumes it.
                xt, Wd = st["xt"], st["W"]
                sgp = cpool.tile([128, 2 * T], bf, tag="sg", name="sg_t", bufs=4)[
                    :, : 2 * Wd
                ]
                for i, (wb, xg0) in enumerate(((12, 2), (17, 0))):
                    gp = psum.tile([128, T], f32, tag="psg", name="psg_t", bufs=2)[
                        :, :Wd
                    ]
                    mm_into(
                        gp,
                        [
                            (wb + 0, xt[:, xg0 * Wd : (xg0 + 1) * Wd]),
                            (wb + 1, xt[:, (xg0 + 1) * Wd : (xg0 + 2) * Wd]),
                        ],
                    )
                    nc.scalar.copy(out=sgp[:, i * Wd : (i + 1) * Wd], in_=gp)
                st["sgp"] = sgp

            def stage_a(st):
                xt, mbt, Wd, yd = st["xt"], st["mbt"], st["W"], st["yd"]

                # pvs[m,k]: x1[k] * mb[m] for m in {v0,v1,v2,s}, k in
                # {x1e_0..2, x1o_0..2} -- ONE 24W DVE op.
                pvs = pvpool.tile([128, 24 * T], bf, tag="pv", name="pv_t")[
                    :, : 24 * Wd
                ]
                nc.vector.tensor_mul(
                    pvs.rearrange("p (m k t) -> p m k t", m=4, k=6),
                    xt[:, 4 * Wd :]
                    .rearrange("p (k t) -> p k t", k=6)
                    .unsqueeze(1)
                    .broadcast_to([128, 4, 6, Wd]),
                    mbt.rearrange("p (c t) -> p c t", c=4)
                    .unsqueeze(2)
                    .broadcast_to([128, 4, 6, Wd]),
                )

                def P(m, k):  # block offset helper
                    o = (m * 6 + k) * Wd
                    return pvs[:, o : o + Wd]

                # ps0 = x0 * s (4 chunks)
                ps0 = pspool.tile([128, 4 * T], bf, tag="ps", name="ps_t")[
                    :, : 4 * Wd
                ]
                nc.vector.tensor_mul(
                    ps0.rearrange("p (c t) -> p c t", c=4),
                    xt[:, : 4 * Wd].rearrange("p (c t) -> p c t", c=4),
                    mbt[:, 3 * Wd : 4 * Wd].unsqueeze(1).broadcast_to([128, 4, Wd]),
                )

                # dots: dta = diag0 + diag1, dotp = dta + diag2 per parity
                # (a=0: 0o dot over x1e, a=1: 0e dot over x1o)
                def dpair(c):
                    # blocks {P(c,c), P(c,c+3)} -> [128, 2, Wd]
                    o = (c * 6 + c) * Wd
                    return pvs[:, o : o + 6 * Wd].rearrange(
                        "p (a k t) -> p a k t", a=2, k=3
                    )[:, :, 0, :]

                dta = cpool.tile([128, 2 * T], bf, tag="dta", name="dta_t", bufs=2)[
                    :, : 2 * Wd
                ]
                dotp = cpool.tile([128, 2 * T], bf, tag="dot", name="dot_t", bufs=2)[
                    :, : 2 * Wd
                ]
                dview = lambda ap: ap.rearrange("p (a t) -> p a t", a=2)
                nc.vector.tensor_add(dview(dta), dpair(0), dpair(1))
                nc.vector.tensor_add(dview(dotp), dview(dta), dpair(2))

                yt = ypool.tile([128, 10 * T], bf, tag="yo", name="y_t")[
                    :, : 10 * Wd
                ]
                # 0e / 0o : both m-chunks in one [2W] psum, single Act copy
                for base, wb, x0c, da in ((0, 0, 0, 1), (2, 6, 2, 0)):
                    pp = psum.tile(
                        [128, 2 * T], f32, tag="ps0", name="ps0_t", bufs=2
                    )[:, : 2 * Wd]
                    for m in range(2):
                        mm_into(
                            pp[:, m * Wd : (m + 1) * Wd],
                            [
                                (wb + 0 * 2 + m, ps0[:, x0c * Wd : (x0c + 1) * Wd]),
                                (
                                    wb + 1 * 2 + m,
                                    ps0[:, (x0c + 1) * Wd : (x0c + 2) * Wd],
                                ),
                                (wb + 2 * 2 + m, dotp[:, da * Wd : (da + 1) * Wd]),
                            ],
                        )
                    nc.scalar.copy(
                        out=yt[:, base * Wd : (base + 2) * Wd], in_=pp
                    )
                if not st["wedge"]:
                    # store 0e+0o as one transfer (wedges store once at
                    # the end of stage_b to minimize Sync issues)
                    nc.sync.dma_start(
                        out=yd[:, : 4 * Wd], in_=yt[:, : 4 * Wd]
                    )
                st.update({"pvs": pvs, "yt": yt})

            def stage_b_dve(st):
                # t3[i,c] = v_c * g_i for both parities in one DVE op
                mbt, sgp, Wd = st["mbt"], st["sgp"], st["W"]
                t3p = cpool.tile([128, 6 * T], bf, tag="t3", name="t3_t", bufs=4)[
                    :, : 6 * Wd
                ]
                nc.vector.tensor_mul(
                    t3p.rearrange("p (i c t) -> p i c t", i=2, c=3),
                    mbt[:, : 3 * Wd]
                    .rearrange("p (c t) -> p c t", c=3)
                    .unsqueeze(1)
                    .broadcast_to([128, 2, 3, Wd]),
                    sgp.rearrange("p (i t) -> p i t", i=2)
                    .unsqueeze(2)
                    .broadcast_to([128, 2, 3, Wd]),
                )
                st["t3p"] = t3p

            def stage_b_pe(st):
                pvs, yt, t3p, Wd, yd = (
                    st["pvs"],
                    st["yt"],
                    st["t3p"],
                    st["W"],
                    st["yd"],
                )

                def P(m, k):
                    o = (m * 6 + k) * Wd
                    return pvs[:, o : o + Wd]

                # out1e: x1o products (k base 3), h over x1e*s (pvs m=3,k=0..2)
                # out1o: x1e products (k base 0), h over x1o*s (pvs m=3,k=3..5)
                for i, (wb, kb, hoff, ob) in enumerate(
                    ((12, 3, 18, 4), (17, 0, 21, 7))
                ):
                    # k+ : x1_a*v_b ; k- : x1_b*v_a  (a=c+1, b=c+2 mod 3)
                    # slice-major accumulation (see module docstring)
                    def contribs(c):
                        a, b = (c + 1) % 3, (c + 2) % 3
                        return [
                            (wb + 3, P(b, kb + a)),
                            (wb + 4, P(a, kb + b)),
                            (22, t3p[:, (i * 3 + c) * Wd : (i * 3 + c + 1) * Wd]),
                            (wb + 2, pvs[:, (hoff + c) * Wd : (hoff + c + 1) * Wd]),
                        ]

                    # components 0,1 share a [2W] psum + one copy; c=2 alone
                    pp = psum.tile(
                        [128, 2 * T], f32, tag="ps1", name="ps1_t", bufs=1
                    )[:, : 2 * Wd]
                    for c in range(2):
                        mm_into(pp[:, c * Wd : (c + 1) * Wd], contribs(c))
                    pc2 = psum.tile([128, T], f32, tag="psg", name="ps1c_t", bufs=2)[
                        :, :Wd
                    ]
                    mm_into(pc2, contribs(2))
                    nc.scalar.copy(out=yt[:, ob * Wd : (ob + 2) * Wd], in_=pp)
                    nc.scalar.copy(
                        out=yt[:, (ob + 2) * Wd : (ob + 3) * Wd], in_=pc2
                    )
                if st["wedge"]:
                    nc.sync.dma_start(out=yd[:, :], in_=yt)
                else:
                    nc.sync.dma_start(
                        out=yd[:, 4 * Wd :], in_=yt[:, 4 * Wd :]
                    )

            # software pipeline: loads prefetched TWO segs ahead (a full
            # tile load is ~4.5us), stage B (t3 + 1e/1o matmuls + store)
            # one seg behind stage A.  The drain-tile wedges' stage_g is
            # hoisted to right after their (batched) load so their sgp ->
            # t3 chain never stalls the DVE at the tail.
            if nmid > 0:
                states = {0: load_mid(0)}
                nc.sync.dma_start(out=wt[:, :], in_=w[:, :])
                if nmid > 1:
                    states[1] = load_mid(1)
                trail_loaded = False
            else:
                states = {jj: st for jj, st in load_wedge_block(0).items()}
                nc.sync.dma_start(out=wt[:, :], in_=w[:, :])
                trail_loaded = True
            for i in range(nseg):
                j = i + 2
                if j < nseg and j not in states:
                    kind, t, _ = segs[j]
                    if kind == "m":
                        states[j] = load_mid(t)
                    elif not trail_loaded:
                        states.update(
                            {nmid + jj: st
                             for jj, st in load_wedge_block(0).items()}
                        )
                        trail_loaded = True
                if trail_loaded and nmid > 0 and i == nmid - 1:
                    # hoist drain-wedge stage_g: their x data landed a seg
                    # ago; sgp then has a full seg of slack before t3
                    for jj in range(NW):
                        if "sgp" not in states[nmid + jj]:
                            stage_g(states[nmid + jj])
                if "sgp" not in states[i]:
                    stage_g(states[i])
                if i >= 1:
                    stage_b_dve(states[i - 1])
                    stage_b_pe(states[i - 1])
                stage_a(states[i])
                if i >= 1:
                    del states[i - 1]
            stage_b_dve(states[nseg - 1])
            stage_b_pe(states[nseg - 1])
    nc.finalize()
    return nc


_PROG_CACHE = {}


def _get_program(Bs):
    if Bs not in _PROG_CACHE:
        _PROG_CACHE[Bs] = _build_program(Bs)
    return _PROG_CACHE[Bs]


def run(inputs, trace=False, **kw):
    in1 = np.asarray(inputs["in1"], np.float32)
    in2 = np.asarray(inputs["in2"], np.float32)
    B = in1.shape[0]
    assert B % (N_CORES * T) == 0, B
    Bs = B // N_CORES

    wpk = _pack_weights(
        np.asarray(inputs["W0e"], np.float32),
        np.asarray(inputs["W0o"], np.float32),
        np.asarray(inputs["W1e"], np.float32),
        np.asarray(inputs["W1o"], np.float32),
    )

    in_maps = []
    for i in range(N_CORES):
        ssl = slice(i * Bs, (i + 1) * Bs)
        xm, s4m, xew, s4ew = _prep_shard(in1[ssl], in2[ssl])
        in_maps.append({"x": xm, "s4": s4m, "xe": xew, "s4e": s4ew, "w": wpk})

    nc = _get_program(Bs)
    res = run_bass_kernel_spmd(nc, in_maps, list(range(N_CORES)), trace=trace, **kw)

    out = np.empty((B, 1280), np.float32)
    for i in range(N_CORES):
        out[i * Bs : (i + 1) * Bs] = _post_shard(
            res.results[i]["y"], res.results[i]["ye"], Bs
        )
    return out, res


def kernel(**inputs):
    out, _ = run(inputs, trace=False)
    return out


# revision 18
# speedup vs baseline: 1.1306x; 1.0360x over previous
"""Trainium2 Bass kernel for the L1 tensor-product problem.

Math (per batch row b):
  out0e = [x0e*s, CG*(x1o.v)] @ W0e * NORM0E
  out0o = [x0o*s, CG*(x1e.v)] @ W0o * NORM0O
  out1e_c = [CG*x0o*v_c, CG*x1e_c*s, CGC*cross(x1o,v)_c] @ W1e * NORM1E
  out1o_c = [CG*x0e*v_c, CG*x1o_c*s, CGC*cross(x1e,v)_c] @ W1o * NORM1O

Kernel strategy (pure data parallel over batch, 8 cores), v5:
  * bf16 wire + matmul dtype; PSUM accumulates fp32 (rel-err ~4e-3,
    budget 2e-2).
  * DVE is the bottleneck engine (~100% busy): all per-row products run
    as 2x-mode bf16 tensor_tensor ops at the hw max of ~1.92 elem/ns,
    and the schedule keeps DVE *elements* at the formulation's minimum
    (38 chunks per tile-column):
      - pvs: ONE 24T op computes x1 x {v0, v1, v2, s} in an [m,k,t]
        layout: x1e*s / x1o*s land contiguous for the h-path matmuls
        and diag/off-diag blocks at AP-addressable offsets for the
        k+/k- matmuls and the dots.
      - dots (2 adds, paired across parities), t3 = v_c*g (6T).
  * Unscaled g = x0?' @ Wg at the head of each PE seg so sgp is ready
    a full seg before the t3 op that consumes it.
  * First/last tile are split into 4 W=128 subtiles so the pipeline
    ramp (first load -> first DVE op) and the tail drain chain are a
    quarter-length.  Edge wedges are packed CONTIGUOUSLY on the host
    (xe/ye/s4e params) -- slicing wedges out of the tile-major layout
    shatters the DMA into 256B packets (measured 40k packets, DMA 86%
    busy, DVE starved at 80%).
  * All DMA in/out APs are 2D contiguous slices; each partition's
    tile-load is one 10KB descriptor run.
  * Multiplier rows (v,s) reach all 128 partitions via a stride-0
    broadcast DMA read (GpSimd is unusable: its SBUF port contends
    with 2-port DVE ops and its tensor ops trip the chip's utilization
    throttle; PE ones-broadcasts would eat the PE headroom).
  * PSUM accumulation is slice-major (see memory: interleaving
    start/stop groups across slices of one psum region is wrong on hw).
"""

import sys

sys.path.insert(0, "/opt/trn_rl_repo")

import numpy as np

import concourse.bass as bass
import concourse.bacc as bacc
import concourse.mybir as mybir
from concourse.bass_utils import run_bass_kernel_spmd
from concourse.tile import TileContext

N_CORES = 8
T = 512  # batch columns per full tile
WEDGE = 256  # subtile width for the last (drain) tile
NW = T // WEDGE  # wedges in the drain tile

# irreps: 256x0e + 256x0o + 128x1e + 128x1o
CG = 1.0 / 3.0**0.5
CGC = 1.0 / 6.0**0.5
NORM0E = (1.0 / 384.0) ** 0.5
NORM0O = (1.0 / 384.0) ** 0.5
NORM1E = (3.0 / 512.0) ** 0.5
NORM1O = (3.0 / 512.0) ** 0.5

_BF16 = None


def _bf16():
    global _BF16
    if _BF16 is None:
        import ml_dtypes

        _BF16 = np.dtype(ml_dtypes.bfloat16)
    return _BF16


def _pack_weights(W0e, W0o, W1e, W1o):
    """Fold constants/signs; 22 lhsT chunks [128,128] side by side.

    Order: 0e (kc0m0,kc0m1,kc1m0,kc1m1,kc2m0,kc2m1), 0o (same 6),
    1e (g0,g1,h,k+,k-), 1o (g0,g1,h,k+,k-), identity.
    """
    W0e = W0e.astype(np.float64) * NORM0E
    W0e[256:] *= CG
    W0o = W0o.astype(np.float64) * NORM0O
    W0o[256:] *= CG
    W1e = W1e.astype(np.float64) * NORM1E
    W1e[:384] *= CG
    W1e[384:] *= CGC
    W1o = W1o.astype(np.float64) * NORM1O
    W1o[:384] *= CG
    W1o[384:] *= CGC
    chunks = []
    for W in (W0e, W0o):  # [384, 256]
        for kc in range(3):
            for mc in range(2):
                chunks.append(W[kc * 128 : (kc + 1) * 128, mc * 128 : (mc + 1) * 128])
    for W in (W1e, W1o):  # [512, 128]
        chunks.append(W[0:128, :])      # g0
        chunks.append(W[128:256, :])    # g1
        chunks.append(W[256:384, :])    # h
        chunks.append(W[384:512, :])    # k+
        chunks.append(-W[384:512, :])   # k-
    chunks.append(np.eye(128, dtype=np.float64))  # 22: identity (combine accum)
    packed = np.concatenate(chunks, axis=1)
    return np.ascontiguousarray(packed.astype(_bf16()))


def _pack_rows(in1_r, in2_r, Wd):
    """Pack a block of Wd rows: -> x [128, 10*Wd] bf16, s4 [4, Wd] bf16.

    Chunk order: 0,1=x0e  2,3=x0o  4+c=x1e_c  7+c=x1o_c.
    Multiplier rows in [v0, v1, v2, s] order.
    """
    dt = _bf16()
    x = np.empty((128, 10, Wd), dt)
    x[:, 0:4] = in1_r[:, 0:512].reshape(Wd, 4, 128).transpose(2, 1, 0)
    x[:, 4:7] = in1_r[:, 512:896].reshape(Wd, 128, 3).transpose(1, 2, 0)
    x[:, 7:10] = in1_r[:, 896:1280].reshape(Wd, 128, 3).transpose(1, 2, 0)
    s4 = np.ascontiguousarray(in2_r[:, [1, 2, 3, 0]].T.astype(dt))
    return np.ascontiguousarray(x.reshape(128, 10 * Wd)), s4


def _prep_shard(in1_s, in2_s):
    """Leading tiles tile-major + trailing (drain) wedges contiguous."""
    Bs = in1_s.shape[0]
    nt = Bs // T
    dt = _bf16()
    nmid = nt - 1 if nt > 1 else 0
    edges = [(Bs - T + j * WEDGE, WEDGE) for j in range(NW)]
    xm = np.empty((max(nmid, 1), 128, 10 * T), dt)
    s4m = np.empty((max(nmid, 1), 4, T), dt)
    for t in range(nmid):
        r = slice(t * T, (t + 1) * T)
        xm[t], s4m[t] = _pack_rows(in1_s[r], in2_s[r], T)
    ne = len(edges)
    xe = np.empty((ne, 128, 10 * WEDGE), dt)
    s4e = np.empty((ne, 4, WEDGE), dt)
    for j, (off, Wd) in enumerate(edges):
        xe[j], s4e[j] = _pack_rows(in1_s[off : off + Wd], in2_s[off : off + Wd], Wd)
    return xm, s4m, xe, s4e


def _unpack_block(yb, Wd):
    """[128, 10*Wd] bf16 -> [Wd, 1280] fp32."""
    yb = np.asarray(yb).reshape(128, 10, Wd).astype(np.float32)
    out = np.empty((Wd, 1280), np.float32)
    out[:, 0:512] = yb[:, 0:4].transpose(2, 1, 0).reshape(Wd, 512)
    out[:, 512:896] = yb[:, 4:7].transpose(2, 0, 1).reshape(Wd, 384)
    out[:, 896:1280] = yb[:, 7:10].transpose(2, 0, 1).reshape(Wd, 384)
    return out


def _post_shard(ym, ye, Bs):
    nt = Bs // T
    out = np.empty((Bs, 1280), np.float32)
    nmid = nt - 1 if nt > 1 else 0
    for t in range(nmid):
        out[t * T : (t + 1) * T] = _unpack_block(ym[t], T)
    for j in range(NW):
        out[Bs - T + j * WEDGE : Bs - T + (j + 1) * WEDGE] = _unpack_block(
            ye[j], WEDGE
        )
    return out


def _build_program(Bs):
    assert Bs % T == 0, (Bs, T)
    nt = Bs // T
    nmid = nt - 1 if nt > 1 else 0
    ne = NW
    bf = mybir.dt.bfloat16
    f32 = mybir.dt.float32

    nc = bacc.Bacc()
    x = nc.declare_dram_parameter("x", [max(nmid, 1), 128, 10 * T], bf, isOutput=False)
    s4 = nc.declare_dram_parameter("s4", [max(nmid, 1), 4, T], bf, isOutput=False)
    xe = nc.declare_dram_parameter("xe", [ne, 128, 10 * WEDGE], bf, isOutput=False)
    s4e = nc.declare_dram_parameter("s4e", [ne, 4, WEDGE], bf, isOutput=False)
    w = nc.declare_dram_parameter("w", [128, 23 * 128], bf, isOutput=False)
    y = nc.declare_dram_parameter("y", [max(nmid, 1), 128, 10 * T], bf, isOutput=True)
    ye = nc.declare_dram_parameter("ye", [ne, 128, 10 * WEDGE], bf, isOutput=True)

    # segment order: full tiles, then drain wedges
    segs = [("m", t, T) for t in range(nmid)]
    segs += [("e", j, WEDGE) for j in range(NW)]
    nseg = len(segs)

    with TileContext(nc) as tc:
        with (
            tc.tile_pool(name="wpool", bufs=1) as wpool,
            tc.tile_pool(name="xpool", bufs=4) as xpool,
            tc.tile_pool(name="mbpool", bufs=4) as mbpool,
            tc.tile_pool(name="pvpool", bufs=2) as pvpool,
            tc.tile_pool(name="pspool", bufs=3) as pspool,
            tc.tile_pool(name="cpool", bufs=2) as cpool,
            tc.tile_pool(name="ypool", bufs=2) as ypool,
            tc.tile_pool(name="psum", bufs=8, space="PSUM") as psum,
        ):
            wt = wpool.tile([128, 23 * 128], bf)

            def W(i):
                return wt[:, i * 128 : (i + 1) * 128]

            def load_wedge_block(j0):
                """ONE set of dma_starts for NW wedges (the per-seg Sync
                issue cost of ~0.6-1.4us per dma_start otherwise starves
                the short wedge segs)."""
                WB = 10 * WEDGE
                xt4 = xpool.tile([128, 10 * T], bf, tag="xt", name="x_t")
                xv = xe[j0 : j0 + NW].rearrange("j p c -> p j c")
                ov = xt4[:, : NW * WB].rearrange("p (j c) -> p j c", j=NW)
                nc.sync.dma_start(out=ov[:, :, 4 * WEDGE :], in_=xv[:, :, 4 * WEDGE :])
                nc.sync.dma_start(out=ov[:, :, : 4 * WEDGE], in_=xv[:, :, : 4 * WEDGE])
                mbt4 = mbpool.tile([128, 4 * T], bf, tag="mb", name="mb_t")
                nc.sync.dma_start(
                    out=mbt4[:, : NW * 4 * WEDGE].rearrange(
                        "p (j c t) -> p j c t", j=NW, c=4
                    ),
                    in_=s4e[j0 : j0 + NW]
                    .unsqueeze(0)
                    .broadcast_to([128, NW, 4, WEDGE]),
                )
                out = {}
                for j in range(NW):
                    out[j] = {
                        "xt": xt4[:, j * WB : (j + 1) * WB],
                        "mbt": mbt4[:, j * 4 * WEDGE : (j + 1) * 4 * WEDGE],
                        "W": WEDGE,
                        "yd": ye[j0 + j],
                        "wedge": True,
                    }
                return out

            def load_mid(t, split=False):
                xt = xpool.tile([128, 10 * T], bf, tag="xt", name="x_t")
                if split:
                    # upper 6 chunks first: the first pvs op needs only
                    # these, so the pipeline ramp starts ~2.6us earlier
                    nc.sync.dma_start(out=xt[:, 4 * T :], in_=x[t, :, 4 * T :])
                    nc.sync.dma_start(out=xt[:, : 4 * T], in_=x[t, :, : 4 * T])
                else:
                    nc.sync.dma_start(out=xt[:, :], in_=x[t])
                mbt = mbpool.tile([128, 4 * T], bf, tag="mb", name="mb_t")
                nc.sync.dma_start(
                    out=mbt[:, :].rearrange("p (c t) -> p c t", c=4),
                    in_=s4[t].unsqueeze(0).broadcast_to([128, 4, T]),
                )
                return {"xt": xt, "mbt": mbt, "W": T, "yd": y[t], "wedge": False}

            def mm_into(p, contribs, first=True, last=True):
                n = len(contribs)
                for i, (wi, rhs) in enumerate(contribs):
                    nc.tensor.matmul(
                        p,
                        W(wi),
                        rhs,
                        start=(first and i == 0),
                        stop=(last and i == n - 1),
                    )

            def stage_g(st):
                # g = x0?' @ Wg (unscaled; only needs xt).  Emitted at the
                # head of each PE seg so sgp is ready a full seg before the
                # t3 op that consumes it.
                xt, Wd = st["xt"], st["W"]
                sgp = cpool.tile([128, 2 * T], bf, tag="sg", name="sg_t", bufs=4)[
                    :, : 2 * Wd
                ]
                for i, (wb, xg0) in enumerate(((12, 2), (17, 0))):
                    gp = psum.tile([128, T], f32, tag="psg", name="psg_t", bufs=2)[
                        :, :Wd
                    ]
                    mm_into(
                        gp,
                        [
                            (wb + 0, xt[:, xg0 * Wd : (xg0 + 1) * Wd]),
                            (wb + 1, xt[:, (xg0 + 1) * Wd : (xg0 + 2) * Wd]),
                        ],
                    )
                    nc.scalar.copy(out=sgp[:, i * Wd : (i + 1) * Wd], in_=gp)
                st["sgp"] = sgp

            def stage_a(st):
                xt, mbt, Wd, yd = st["xt"], st["mbt"], st["W"], st["yd"]

                # pvs[m,k]: x1[k] * mb[m] for m in {v0,v1,v2,s}, k in
                # {x1e_0..2, x1o_0..2} -- ONE 24W DVE op.
                pvs = pvpool.tile([128, 24 * T], bf, tag="pv", name="pv_t")[
                    :, : 24 * Wd
                ]
                nc.vector.tensor_mul(
                    pvs.rearrange("p (m k t) -> p m k t", m=4, k=6),
                    xt[:, 4 * Wd :]
                    .rearrange("p (k t) -> p k t", k=6)
                    .unsqueeze(1)
                    .broadcast_to([128, 4, 6, Wd]),
                    mbt.rearrange("p (c t) -> p c t", c=4)
                    .unsqueeze(2)
                    .broadcast_to([128, 4, 6, Wd]),
                )

                def P(m, k):  # block offset helper
                    o = (m * 6 + k) * Wd
                    return pvs[:, o : o + Wd]

                # ps0 = x0 * s (4 chunks)
                ps0 = pspool.tile([128, 4 * T], bf, tag="ps", name="ps_t")[
                    :, : 4 * Wd
                ]
                nc.vector.tensor_mul(
                    ps0.rearrange("p (c t) -> p c t", c=4),
                    xt[:, : 4 * Wd].rearrange("p (c t) -> p c t", c=4),
                    mbt[:, 3 * Wd : 4 * Wd].unsqueeze(1).broadcast_to([128, 4, Wd]),
                )

                # dots: dta = diag0 + diag1, dotp = dta + diag2 per parity
                # (a=0: 0o dot over x1e, a=1: 0e dot over x1o)
                def dpair(c):
                    # blocks {P(c,c), P(c,c+3)} -> [128, 2, Wd]
                    o = (c * 6 + c) * Wd
                    return pvs[:, o : o + 6 * Wd].rearrange(
                        "p (a k t) -> p a k t", a=2, k=3
                    )[:, :, 0, :]

                dta = cpool.tile([128, 2 * T], bf, tag="dta", name="dta_t", bufs=2)[
                    :, : 2 * Wd
                ]
                dview = lambda ap: ap.rearrange("p (a t) -> p a t", a=2)
                nc.vector.tensor_add(dview(dta), dpair(0), dpair(1))

                yt = ypool.tile([128, 10 * T], bf, tag="yo", name="y_t")[
                    :, : 10 * Wd
                ]
                # 0e / 0o : both m-chunks in one [2W] psum, single Act copy
                for base, wb, x0c, da in ((0, 0, 0, 1), (2, 6, 2, 0)):
                    # dot = dta + diag2 distributed into the accumulation
                    # (diag2: 0e = P(2,5) over x1o, 0o = P(2,2) over x1e)
                    dg2 = P(2, 5) if base == 0 else P(2, 2)
                    pp = psum.tile(
                        [128, 2 * T], f32, tag="ps0", name="ps0_t", bufs=2
                    )[:, : 2 * Wd]
                    for m in range(2):
                        mm_into(
                            pp[:, m * Wd : (m + 1) * Wd],
                            [
                                (wb + 0 * 2 + m, ps0[:, x0c * Wd : (x0c + 1) * Wd]),
                                (
                                    wb + 1 * 2 + m,
                                    ps0[:, (x0c + 1) * Wd : (x0c + 2) * Wd],
                                ),
                                (wb + 2 * 2 + m, dta[:, da * Wd : (da + 1) * Wd]),
                                (wb + 2 * 2 + m, dg2),
                            ],
                        )
                    nc.scalar.copy(
                        out=yt[:, base * Wd : (base + 2) * Wd], in_=pp
                    )
                # store 0e+0o as one transfer
                nc.sync.dma_start(out=yd[:, : 4 * Wd], in_=yt[:, : 4 * Wd])
                st.update({"pvs": pvs, "yt": yt})

            def stage_b_dve(st):
                # t3[i,c] = v_c * g_i for both parities in one DVE op
                mbt, sgp, Wd = st["mbt"], st["sgp"], st["W"]
                t3p = cpool.tile([128, 6 * T], bf, tag="t3", name="t3_t", bufs=4)[
                    :, : 6 * Wd
                ]
                nc.vector.tensor_mul(
                    t3p.rearrange("p (i c t) -> p i c t", i=2, c=3),
                    mbt[:, : 3 * Wd]
                    .rearrange("p (c t) -> p c t", c=3)
                    .unsqueeze(1)
                    .broadcast_to([128, 2, 3, Wd]),
                    sgp.rearrange("p (i t) -> p i t", i=2)
                    .unsqueeze(2)
                    .broadcast_to([128, 2, 3, Wd]),
                )
                st["t3p"] = t3p

            def stage_b_pe(st):
                pvs, yt, t3p, Wd, yd = (
                    st["pvs"],
                    st["yt"],
                    st["t3p"],
                    st["W"],
                    st["yd"],
                )

                def P(m, k):
                    o = (m * 6 + k) * Wd
                    return pvs[:, o : o + Wd]

                # out1e: x1o products (k base 3), h over x1e*s (pvs m=3,k=0..2)
                # out1o: x1e products (k base 0), h over x1o*s (pvs m=3,k=3..5)
                for i, (wb, kb, hoff, ob) in enumerate(
                    ((12, 3, 18, 4), (17, 0, 21, 7))
                ):
                    # k+ : x1_a*v_b ; k- : x1_b*v_a  (a=c+1, b=c+2 mod 3)
                    # slice-major accumulation (see module docstring)
                    def contribs(c):
                        a, b = (c + 1) % 3, (c + 2) % 3
                        return [
                            (wb + 3, P(b, kb + a)),
                            (wb + 4, P(a, kb + b)),
                            (22, t3p[:, (i * 3 + c) * Wd : (i * 3 + c + 1) * Wd]),
                            (wb + 2, pvs[:, (hoff + c) * Wd : (hoff + c + 1) * Wd]),
                        ]

                    # components 0,1 share a [2W] psum + one copy; c=2 alone
                    pp = psum.tile(
                        [128, 2 * T], f32, tag="ps1", name="ps1_t", bufs=1
                    )[:, : 2 * Wd]
                    for c in range(2):
                        mm_into(pp[:, c * Wd : (c + 1) * Wd], contribs(c))
                    pc2 = psum.tile([128, T], f32, tag="psg", name="ps1c_t", bufs=2)[
                        :, :Wd
                    ]
                    mm_into(pc2, contribs(2))
                    nc.scalar.copy(out=yt[:, ob * Wd : (ob + 2) * Wd], in_=pp)
                    nc.scalar.copy(
                        out=yt[:, (ob + 2) * Wd : (ob + 3) * Wd], in_=pc2
                    )
                    nc.sync.dma_start(
                        out=yd[:, ob * Wd : (ob + 3) * Wd],
                        in_=yt[:, ob * Wd : (ob + 3) * Wd],
                    )

            # software pipeline: loads prefetched TWO segs ahead (a full
            # tile load is ~4.5us), stage B (t3 + 1e/1o matmuls + store)
            # one seg behind stage A.  The drain-tile wedges' stage_g is
            # hoisted to right after their (batched) load so their sgp ->
            # t3 chain never stalls the DVE at the tail.
            if nmid > 0:
                states = {0: load_mid(0, split=True)}
                nc.sync.dma_start(out=wt[:, :], in_=w[:, :])
                if nmid > 1:
                    states[1] = load_mid(1)
                trail_loaded = False
            else:
                states = {jj: st for jj, st in load_wedge_block(0).items()}
                nc.sync.dma_start(out=wt[:, :], in_=w[:, :])
                trail_loaded = True
            for i in range(nseg):
                j = i + 2
                if j < nseg and j not in states:
                    kind, t, _ = segs[j]
                    if kind == "m":
                        states[j] = load_mid(t)
                    elif not trail_loaded:
                        states.update(
                            {nmid + jj: st
                             for jj, st in load_wedge_block(0).items()}
                        )
                        trail_loaded = True
                if trail_loaded and nmid > 0 and i == nmid - 1:
                    # hoist drain-wedge stage_g: their x data landed a seg
                    # ago; sgp then has a full seg of slack before t3
                    for jj in range(NW):
                        if "sgp" not in states[nmid + jj]:
                            stage_g(states[nmid + jj])
                if "sgp" not in states[i]:
                    stage_g(states[i])
                if i >= 1:
                    stage_b_dve(states[i - 1])
                    stage_b_pe(states[i - 1])
                stage_a(states[i])
                if i >= 1:
                    del states[i - 1]
            stage_b_dve(states[nseg - 1])
            stage_b_pe(states[nseg - 1])
    nc.finalize()
    return nc


_PROG_CACHE = {}


def _get_program(Bs):
    if Bs not in _PROG_CACHE:
        _PROG_CACHE[Bs] = _build_program(Bs)
    return _PROG_CACHE[Bs]


def run(inputs, trace=False, **kw):
    in1 = np.asarray(inputs["in1"], np.float32)
    in2 = np.asarray(inputs["in2"], np.float32)
    B = in1.shape[0]
    assert B % (N_CORES * T) == 0, B
    Bs = B // N_CORES

    wpk = _pack_weights(
        np.asarray(inputs["W0e"], np.float32),
        np.asarray(inputs["W0o"], np.float32),
        np.asarray(inputs["W1e"], np.float32),
        np.asarray(inputs["W1o"], np.float32),
    )

    in_maps = []
    for i in range(N_CORES):
        ssl = slice(i * Bs, (i + 1) * Bs)
        xm, s4m, xew, s4ew = _prep_shard(in1[ssl], in2[ssl])
        in_maps.append({"x": xm, "s4": s4m, "xe": xew, "s4e": s4ew, "w": wpk})

    nc = _get_program(Bs)
    res = run_bass_kernel_spmd(nc, in_maps, list(range(N_CORES)), trace=trace, **kw)

    out = np.empty((B, 1280), np.float32)
    for i in range(N_CORES):
        out[i * Bs : (i + 1) * Bs] = _post_shard(
            res.results[i]["y"], res.results[i]["ye"], Bs
        )
    return out, res


def kernel(**inputs):
    out, _ = run(inputs, trace=False)
    return out


# revision 19
# speedup vs baseline: 1.1398x; 1.0081x over previous
"""Trainium2 Bass kernel for the L1 tensor-product problem.

Math (per batch row b):
  out0e = [x0e*s, CG*(x1o.v)] @ W0e * NORM0E
  out0o = [x0o*s, CG*(x1e.v)] @ W0o * NORM0O
  out1e_c = [CG*x0o*v_c, CG*x1e_c*s, CGC*cross(x1o,v)_c] @ W1e * NORM1E
  out1o_c = [CG*x0e*v_c, CG*x1o_c*s, CGC*cross(x1e,v)_c] @ W1o * NORM1O

Kernel strategy (pure data parallel over batch, 8 cores), v5:
  * bf16 wire + matmul dtype; PSUM accumulates fp32 (rel-err ~4e-3,
    budget 2e-2).
  * DVE is the bottleneck engine (~100% busy): all per-row products run
    as 2x-mode bf16 tensor_tensor ops at the hw max of ~1.92 elem/ns,
    and the schedule keeps DVE *elements* at the formulation's minimum
    (38 chunks per tile-column):
      - pvs: ONE 24T op computes x1 x {v0, v1, v2, s} in an [m,k,t]
        layout: x1e*s / x1o*s land contiguous for the h-path matmuls
        and diag/off-diag blocks at AP-addressable offsets for the
        k+/k- matmuls and the dots.
      - dots (2 adds, paired across parities), t3 = v_c*g (6T).
  * Unscaled g = x0?' @ Wg at the head of each PE seg so sgp is ready
    a full seg before the t3 op that consumes it.
  * First/last tile are split into 4 W=128 subtiles so the pipeline
    ramp (first load -> first DVE op) and the tail drain chain are a
    quarter-length.  Edge wedges are packed CONTIGUOUSLY on the host
    (xe/ye/s4e params) -- slicing wedges out of the tile-major layout
    shatters the DMA into 256B packets (measured 40k packets, DMA 86%
    busy, DVE starved at 80%).
  * All DMA in/out APs are 2D contiguous slices; each partition's
    tile-load is one 10KB descriptor run.
  * Multiplier rows (v,s) reach all 128 partitions via a stride-0
    broadcast DMA read (GpSimd is unusable: its SBUF port contends
    with 2-port DVE ops and its tensor ops trip the chip's utilization
    throttle; PE ones-broadcasts would eat the PE headroom).
  * PSUM accumulation is slice-major (see memory: interleaving
    start/stop groups across slices of one psum region is wrong on hw).
"""

import sys

sys.path.insert(0, "/opt/trn_rl_repo")

import numpy as np

import concourse.bass as bass
import concourse.bacc as bacc
import concourse.mybir as mybir
from concourse.bass_utils import run_bass_kernel_spmd
from concourse.tile import TileContext

N_CORES = 8
T = 512  # batch columns per full tile
WEDGE = 256  # subtile width for the last (drain) tile
NW = T // WEDGE  # wedges in the drain tile

# irreps: 256x0e + 256x0o + 128x1e + 128x1o
CG = 1.0 / 3.0**0.5
CGC = 1.0 / 6.0**0.5
NORM0E = (1.0 / 384.0) ** 0.5
NORM0O = (1.0 / 384.0) ** 0.5
NORM1E = (3.0 / 512.0) ** 0.5
NORM1O = (3.0 / 512.0) ** 0.5

_BF16 = None


def _bf16():
    global _BF16
    if _BF16 is None:
        import ml_dtypes

        _BF16 = np.dtype(ml_dtypes.bfloat16)
    return _BF16


def _pack_weights(W0e, W0o, W1e, W1o):
    """Fold constants/signs; 22 lhsT chunks [128,128] side by side.

    Order: 0e (kc0m0,kc0m1,kc1m0,kc1m1,kc2m0,kc2m1), 0o (same 6),
    1e (g0,g1,h,k+,k-), 1o (g0,g1,h,k+,k-), identity.
    """
    W0e = W0e.astype(np.float64) * NORM0E
    W0e[256:] *= CG
    W0o = W0o.astype(np.float64) * NORM0O
    W0o[256:] *= CG
    W1e = W1e.astype(np.float64) * NORM1E
    W1e[:384] *= CG
    W1e[384:] *= CGC
    W1o = W1o.astype(np.float64) * NORM1O
    W1o[:384] *= CG
    W1o[384:] *= CGC
    chunks = []
    for W in (W0e, W0o):  # [384, 256]
        for kc in range(3):
            for mc in range(2):
                chunks.append(W[kc * 128 : (kc + 1) * 128, mc * 128 : (mc + 1) * 128])
    for W in (W1e, W1o):  # [512, 128]
        chunks.append(W[0:128, :])      # g0
        chunks.append(W[128:256, :])    # g1
        chunks.append(W[256:384, :])    # h
        chunks.append(W[384:512, :])    # k+
        chunks.append(-W[384:512, :])   # k-
    chunks.append(np.eye(128, dtype=np.float64))  # 22: identity (combine accum)
    packed = np.concatenate(chunks, axis=1)
    return np.ascontiguousarray(packed.astype(_bf16()))


def _pack_rows(in1_r, in2_r, Wd):
    """Pack a block of Wd rows: -> x [128, 10*Wd] bf16, s4 [4, Wd] bf16.

    Chunk order: 0,1=x0e  2,3=x0o  4+c=x1e_c  7+c=x1o_c.
    Multiplier rows in [v0, v1, v2, s] order.
    """
    dt = _bf16()
    x = np.empty((128, 10, Wd), dt)
    x[:, 0:4] = in1_r[:, 0:512].reshape(Wd, 4, 128).transpose(2, 1, 0)
    x[:, 4:7] = in1_r[:, 512:896].reshape(Wd, 128, 3).transpose(1, 2, 0)
    x[:, 7:10] = in1_r[:, 896:1280].reshape(Wd, 128, 3).transpose(1, 2, 0)
    s4 = np.ascontiguousarray(in2_r[:, [1, 2, 3, 0]].T.astype(dt))
    return np.ascontiguousarray(x.reshape(128, 10 * Wd)), s4


def _prep_shard(in1_s, in2_s):
    """Middle tiles tile-major + leading/trailing wedges contiguous."""
    Bs = in1_s.shape[0]
    nt = Bs // T
    dt = _bf16()
    nmid = max(nt - 2, 0)
    edges = [(j * WEDGE, WEDGE) for j in range(NW)]
    if nt > 1:
        edges += [(Bs - T + j * WEDGE, WEDGE) for j in range(NW)]
    xm = np.empty((max(nmid, 1), 128, 10 * T), dt)
    s4m = np.empty((max(nmid, 1), 4, T), dt)
    for t in range(nmid):
        r = slice(T + t * T, T + (t + 1) * T)
        xm[t], s4m[t] = _pack_rows(in1_s[r], in2_s[r], T)
    ne = len(edges)
    xe = np.empty((ne, 128, 10 * WEDGE), dt)
    s4e = np.empty((ne, 4, WEDGE), dt)
    for j, (off, Wd) in enumerate(edges):
        xe[j], s4e[j] = _pack_rows(in1_s[off : off + Wd], in2_s[off : off + Wd], Wd)
    return xm, s4m, xe, s4e


def _unpack_block(yb, Wd):
    """[128, 10*Wd] bf16 -> [Wd, 1280] fp32."""
    yb = np.asarray(yb).reshape(128, 10, Wd).astype(np.float32)
    out = np.empty((Wd, 1280), np.float32)
    out[:, 0:512] = yb[:, 0:4].transpose(2, 1, 0).reshape(Wd, 512)
    out[:, 512:896] = yb[:, 4:7].transpose(2, 0, 1).reshape(Wd, 384)
    out[:, 896:1280] = yb[:, 7:10].transpose(2, 0, 1).reshape(Wd, 384)
    return out


def _post_shard(ym, ye, Bs):
    nt = Bs // T
    out = np.empty((Bs, 1280), np.float32)
    nmid = max(nt - 2, 0)
    for t in range(nmid):
        out[T + t * T : T + (t + 1) * T] = _unpack_block(ym[t], T)
    for j in range(NW):
        out[j * WEDGE : (j + 1) * WEDGE] = _unpack_block(ye[j], WEDGE)
    if nt > 1:
        for j in range(NW):
            out[Bs - T + j * WEDGE : Bs - T + (j + 1) * WEDGE] = _unpack_block(
                ye[NW + j], WEDGE
            )
    return out


def _build_program(Bs):
    assert Bs % T == 0, (Bs, T)
    nt = Bs // T
    nmid = max(nt - 2, 0)
    ne = NW if nt == 1 else 2 * NW
    bf = mybir.dt.bfloat16
    f32 = mybir.dt.float32

    nc = bacc.Bacc()
    x = nc.declare_dram_parameter("x", [max(nmid, 1), 128, 10 * T], bf, isOutput=False)
    s4 = nc.declare_dram_parameter("s4", [max(nmid, 1), 4, T], bf, isOutput=False)
    xe = nc.declare_dram_parameter("xe", [ne, 128, 10 * WEDGE], bf, isOutput=False)
    s4e = nc.declare_dram_parameter("s4e", [ne, 4, WEDGE], bf, isOutput=False)
    w = nc.declare_dram_parameter("w", [128, 23 * 128], bf, isOutput=False)
    y = nc.declare_dram_parameter("y", [max(nmid, 1), 128, 10 * T], bf, isOutput=True)
    ye = nc.declare_dram_parameter("ye", [ne, 128, 10 * WEDGE], bf, isOutput=True)

    # segment order: leading wedges, full tiles, drain wedges
    segs = [("e", j, WEDGE) for j in range(NW)]
    segs += [("m", t, T) for t in range(nmid)]
    if nt > 1:
        segs += [("e", NW + j, WEDGE) for j in range(NW)]
    nseg = len(segs)

    with TileContext(nc) as tc:
        with (
            tc.tile_pool(name="wpool", bufs=1) as wpool,
            tc.tile_pool(name="xpool", bufs=4) as xpool,
            tc.tile_pool(name="mbpool", bufs=4) as mbpool,
            tc.tile_pool(name="pvpool", bufs=3) as pvpool,
            tc.tile_pool(name="pspool", bufs=2) as pspool,
            tc.tile_pool(name="cpool", bufs=2) as cpool,
            tc.tile_pool(name="ypool", bufs=2) as ypool,
            tc.tile_pool(name="psum", bufs=8, space="PSUM") as psum,
        ):
            wt = wpool.tile([128, 23 * 128], bf)

            def W(i):
                return wt[:, i * 128 : (i + 1) * 128]

            def load_wedge_block(j0, lead=False):
                """Batched dma_starts for NW wedges (the per-seg Sync
                issue cost of ~0.6-1.4us per dma_start otherwise starves
                the short wedge segs).  lead=True prioritizes wedge 0's
                multiplier + upper chunks so the first pvs can start as
                early as possible."""
                WB = 10 * WEDGE
                xt4 = xpool.tile([128, 10 * T], bf, tag="xt", name="x_t")
                xv = xe[j0 : j0 + NW].rearrange("j p c -> p j c")
                ov = xt4[:, : NW * WB].rearrange("p (j c) -> p j c", j=NW)
                mbt4 = mbpool.tile([128, 4 * T], bf, tag="mb", name="mb_t")
                mv = mbt4[:, : NW * 4 * WEDGE].rearrange(
                    "p (j c t) -> p j c t", j=NW, c=4
                )
                sv = s4e[j0 : j0 + NW].unsqueeze(0).broadcast_to(
                    [128, NW, 4, WEDGE]
                )
                if lead:
                    for j in range(NW):
                        nc.sync.dma_start(out=mv[:, j], in_=sv[:, j])
                        nc.sync.dma_start(
                            out=ov[:, j, 4 * WEDGE :], in_=xv[:, j, 4 * WEDGE :]
                        )
                    nc.sync.dma_start(
                        out=ov[:, :, : 4 * WEDGE], in_=xv[:, :, : 4 * WEDGE]
                    )
                else:
                    nc.sync.dma_start(
                        out=ov[:, :, 4 * WEDGE :], in_=xv[:, :, 4 * WEDGE :]
                    )
                    nc.sync.dma_start(
                        out=ov[:, :, : 4 * WEDGE], in_=xv[:, :, : 4 * WEDGE]
                    )
                    nc.sync.dma_start(out=mv, in_=sv)
                out = {}
                for j in range(NW):
                    out[j] = {
                        "xt": xt4[:, j * WB : (j + 1) * WB],
                        "mbt": mbt4[:, j * 4 * WEDGE : (j + 1) * 4 * WEDGE],
                        "W": WEDGE,
                        "yd": ye[j0 + j],
                        "wedge": True,
                    }
                return out

            def load_mid(t):
                xt = xpool.tile([128, 10 * T], bf, tag="xt", name="x_t")
                nc.sync.dma_start(out=xt[:, :], in_=x[t])
                mbt = mbpool.tile([128, 4 * T], bf, tag="mb", name="mb_t")
                nc.sync.dma_start(
                    out=mbt[:, :].rearrange("p (c t) -> p c t", c=4),
                    in_=s4[t].unsqueeze(0).broadcast_to([128, 4, T]),
                )
                return {"xt": xt, "mbt": mbt, "W": T, "yd": y[t], "wedge": False}

            def mm_into(p, contribs, first=True, last=True):
                n = len(contribs)
                for i, (wi, rhs) in enumerate(contribs):
                    nc.tensor.matmul(
                        p,
                        W(wi),
                        rhs,
                        start=(first and i == 0),
                        stop=(last and i == n - 1),
                    )

            def stage_g(st):
                # g = x0?' @ Wg (unscaled; only needs xt).  Emitted at the
                # head of each PE seg so sgp is ready a full seg before the
                # t3 op that consumes it.
                xt, Wd = st["xt"], st["W"]
                sgp = cpool.tile([128, 2 * T], bf, tag="sg", name="sg_t", bufs=3)[
                    :, : 2 * Wd
                ]
                for i, (wb, xg0) in enumerate(((12, 2), (17, 0))):
                    gp = psum.tile([128, T], f32, tag="psg", name="psg_t", bufs=2)[
                        :, :Wd
                    ]
                    mm_into(
                        gp,
                        [
                            (wb + 0, xt[:, xg0 * Wd : (xg0 + 1) * Wd]),
                            (wb + 1, xt[:, (xg0 + 1) * Wd : (xg0 + 2) * Wd]),
                        ],
                    )
                    nc.scalar.copy(out=sgp[:, i * Wd : (i + 1) * Wd], in_=gp)
                st["sgp"] = sgp

            def stage_a(st):
                xt, mbt, Wd, yd = st["xt"], st["mbt"], st["W"], st["yd"]

                # pvs[m,k]: x1[k] * mb[m] for m in {v0,v1,v2,s}, k in
                # {x1e_0..2, x1o_0..2} -- ONE 24W DVE op.
                pvs = pvpool.tile([128, 24 * T], bf, tag="pv", name="pv_t")[
                    :, : 24 * Wd
                ]
                nc.vector.tensor_mul(
                    pvs.rearrange("p (m k t) -> p m k t", m=4, k=6),
                    xt[:, 4 * Wd :]
                    .rearrange("p (k t) -> p k t", k=6)
                    .unsqueeze(1)
                    .broadcast_to([128, 4, 6, Wd]),
                    mbt.rearrange("p (c t) -> p c t", c=4)
                    .unsqueeze(2)
                    .broadcast_to([128, 4, 6, Wd]),
                )

                def P(m, k):  # block offset helper
                    o = (m * 6 + k) * Wd
                    return pvs[:, o : o + Wd]

                # ps0 = x0 * s (4 chunks)
                ps0 = pspool.tile([128, 4 * T], bf, tag="ps", name="ps_t")[
                    :, : 4 * Wd
                ]
                nc.vector.tensor_mul(
                    ps0.rearrange("p (c t) -> p c t", c=4),
                    xt[:, : 4 * Wd].rearrange("p (c t) -> p c t", c=4),
                    mbt[:, 3 * Wd : 4 * Wd].unsqueeze(1).broadcast_to([128, 4, Wd]),
                )

                # dots: dta = diag0 + diag1, dotp = dta + diag2 per parity
                # (a=0: 0o dot over x1e, a=1: 0e dot over x1o)
                def dpair(c):
                    # blocks {P(c,c), P(c,c+3)} -> [128, 2, Wd]
                    o = (c * 6 + c) * Wd
                    return pvs[:, o : o + 6 * Wd].rearrange(
                        "p (a k t) -> p a k t", a=2, k=3
                    )[:, :, 0, :]

                dta = cpool.tile([128, 2 * T], bf, tag="dta", name="dta_t", bufs=2)[
                    :, : 2 * Wd
                ]
                dview = lambda ap: ap.rearrange("p (a t) -> p a t", a=2)
                nc.vector.tensor_add(dview(dta), dpair(0), dpair(1))

                yt = ypool.tile([128, 10 * T], bf, tag="yo", name="y_t")[
                    :, : 10 * Wd
                ]
                # 0e / 0o : both m-chunks in one [2W] psum, single Act copy
                for base, wb, x0c, da in ((0, 0, 0, 1), (2, 6, 2, 0)):
                    # dot = dta + diag2 distributed into the accumulation
                    # (diag2: 0e = P(2,5) over x1o, 0o = P(2,2) over x1e)
                    dg2 = P(2, 5) if base == 0 else P(2, 2)
                    pp = psum.tile(
                        [128, 2 * T], f32, tag="ps0", name="ps0_t", bufs=2
                    )[:, : 2 * Wd]
                    for m in range(2):
                        mm_into(
                            pp[:, m * Wd : (m + 1) * Wd],
                            [
                                (wb + 0 * 2 + m, ps0[:, x0c * Wd : (x0c + 1) * Wd]),
                                (
                                    wb + 1 * 2 + m,
                                    ps0[:, (x0c + 1) * Wd : (x0c + 2) * Wd],
                                ),
                                (wb + 2 * 2 + m, dta[:, da * Wd : (da + 1) * Wd]),
                                (wb + 2 * 2 + m, dg2),
                            ],
                        )
                    nc.scalar.copy(
                        out=yt[:, base * Wd : (base + 2) * Wd], in_=pp
                    )
                # store 0e+0o as one transfer
                nc.sync.dma_start(out=yd[:, : 4 * Wd], in_=yt[:, : 4 * Wd])
                st.update({"pvs": pvs, "yt": yt})

            def stage_b_dve(st):
                # t3[i,c] = v_c * g_i for both parities in one DVE op
                mbt, sgp, Wd = st["mbt"], st["sgp"], st["W"]
                t3p = cpool.tile([128, 6 * T], bf, tag="t3", name="t3_t", bufs=3)[
                    :, : 6 * Wd
                ]
                nc.vector.tensor_mul(
                    t3p.rearrange("p (i c t) -> p i c t", i=2, c=3),
                    mbt[:, : 3 * Wd]
                    .rearrange("p (c t) -> p c t", c=3)
                    .unsqueeze(1)
                    .broadcast_to([128, 2, 3, Wd]),
                    sgp.rearrange("p (i t) -> p i t", i=2)
                    .unsqueeze(2)
                    .broadcast_to([128, 2, 3, Wd]),
                )
                st["t3p"] = t3p

            def stage_b_pe(st):
                pvs, yt, t3p, Wd, yd = (
                    st["pvs"],
                    st["yt"],
                    st["t3p"],
                    st["W"],
                    st["yd"],
                )

                def P(m, k):
                    o = (m * 6 + k) * Wd
                    return pvs[:, o : o + Wd]

                # out1e: x1o products (k base 3), h over x1e*s (pvs m=3,k=0..2)
                # out1o: x1e products (k base 0), h over x1o*s (pvs m=3,k=3..5)
                for i, (wb, kb, hoff, ob) in enumerate(
                    ((12, 3, 18, 4), (17, 0, 21, 7))
                ):
                    # k+ : x1_a*v_b ; k- : x1_b*v_a  (a=c+1, b=c+2 mod 3)
                    # slice-major accumulation (see module docstring)
                    def contribs(c):
                        a, b = (c + 1) % 3, (c + 2) % 3
                        return [
                            (wb + 3, P(b, kb + a)),
                            (wb + 4, P(a, kb + b)),
                            (22, t3p[:, (i * 3 + c) * Wd : (i * 3 + c + 1) * Wd]),
                            (wb + 2, pvs[:, (hoff + c) * Wd : (hoff + c + 1) * Wd]),
                        ]

                    # components 0,1 share a [2W] psum + one copy; c=2 alone
                    pp = psum.tile(
                        [128, 2 * T], f32, tag="ps1", name="ps1_t", bufs=1
                    )[:, : 2 * Wd]
                    for c in range(2):
                        mm_into(pp[:, c * Wd : (c + 1) * Wd], contribs(c))
                    pc2 = psum.tile([128, T], f32, tag="psg", name="ps1c_t", bufs=2)[
                        :, :Wd
                    ]
                    mm_into(pc2, contribs(2))
                    nc.scalar.copy(out=yt[:, ob * Wd : (ob + 2) * Wd], in_=pp)
                    nc.scalar.copy(
                        out=yt[:, (ob + 2) * Wd : (ob + 3) * Wd], in_=pc2
                    )
                    nc.sync.dma_start(
                        out=yd[:, ob * Wd : (ob + 3) * Wd],
                        in_=yt[:, ob * Wd : (ob + 3) * Wd],
                    )

            # software pipeline: loads prefetched TWO segs ahead (a full
            # tile load is ~4.5us), stage B (t3 + 1e/1o matmuls + store)
            # one seg behind stage A.  The drain-tile wedges' stage_g is
            # hoisted to right after their (batched) load so their sgp ->
            # t3 chain never stalls the DVE at the tail.
            states = {jj: st for jj, st in load_wedge_block(0, lead=True).items()}
            nc.sync.dma_start(out=wt[:, :], in_=w[:, :])
            trail_loaded = nt == 1
            ntrail0 = NW + nmid  # seg index of the first drain wedge
            for i in range(nseg):
                j = i + 2
                if j < nseg and j not in states:
                    kind, t, _ = segs[j]
                    if kind == "m":
                        states[j] = load_mid(t)
                    elif not trail_loaded:
                        states.update(
                            {ntrail0 + jj: st
                             for jj, st in load_wedge_block(NW).items()}
                        )
                        trail_loaded = True
                if trail_loaded and nt > 1 and i == ntrail0 - 1:
                    # hoist drain-wedge stage_g: their x data landed a seg
                    # ago; sgp then has a full seg of slack before t3
                    for jj in range(NW):
                        if "sgp" not in states.get(ntrail0 + jj, {"sgp": 1}):
                            stage_g(states[ntrail0 + jj])
                if "sgp" not in states[i]:
                    stage_g(states[i])
                if i >= 1:
                    stage_b_dve(states[i - 1])
                    stage_b_pe(states[i - 1])
                stage_a(states[i])
                if i >= 1:
                    del states[i - 1]
            stage_b_dve(states[nseg - 1])
            stage_b_pe(states[nseg - 1])
    nc.finalize()
    return nc


_PROG_CACHE = {}


def _get_program(Bs):
    if Bs not in _PROG_CACHE:
        _PROG_CACHE[Bs] = _build_program(Bs)
    return _PROG_CACHE[Bs]


def run(inputs, trace=False, **kw):
    in1 = np.asarray(inputs["in1"], np.float32)
    in2 = np.asarray(inputs["in2"], np.float32)
    B = in1.shape[0]
    assert B % (N_CORES * T) == 0, B
    Bs = B // N_CORES

    wpk = _pack_weights(
        np.asarray(inputs["W0e"], np.float32),
        np.asarray(inputs["W0o"], np.float32),
        np.asarray(inputs["W1e"], np.float32),
        np.asarray(inputs["W1o"], np.float32),
    )

    in_maps = []
    for i in range(N_CORES):
        ssl = slice(i * Bs, (i + 1) * Bs)
        xm, s4m, xew, s4ew = _prep_shard(in1[ssl], in2[ssl])
        in_maps.append({"x": xm, "s4": s4m, "xe": xew, "s4e": s4ew, "w": wpk})

    nc = _get_program(Bs)
    res = run_bass_kernel_spmd(nc, in_maps, list(range(N_CORES)), trace=trace, **kw)

    out = np.empty((B, 1280), np.float32)
    for i in range(N_CORES):
        out[i * Bs : (i + 1) * Bs] = _post_shard(
            res.results[i]["y"], res.results[i]["ye"], Bs
        )
    return out, res


def kernel(**inputs):
    out, _ = run(inputs, trace=False)
    return out
